# revision 8
# baseline (speedup 1.0000x reference)
"""DeepSeek-V2 MLA attention (weight-absorbed) on 8 Trainium2 NeuronCores.

Sharding: tensor-parallel over the 128 heads (16 heads/core).  The q
LoRA projection (hidden @ Wqa.T) is sharded over the Q_LORA output dim
and AllGathered; the per-head attention runs fully local; the output
projection partials are summed with a ReduceScatter over the token axis
and the 8 shards are concatenated on the host.

Math restructuring vs the reference (exactly associativity-equivalent):
  - q_nope = (q @ qb_nope.T) @ q_absorb          (factor through the 128-dim)
  - o      = softmax(l) @ (ckv @ out_absorb.T)   (decompress V, 128-dim)
  - out    = concat_h(o_h) @ Wo.T                (plain o_proj)
  - rmsnorm's per-token scale and 1/sqrt(192) are folded into q;
    qa_ln_w is folded into Wqb; the RoPE interleave permutation is
    folded into the rope rows of Wqb; softmax skips the max-subtraction
    (logits are O(3) for this problem) and normalizes o after PV.
"""

import math
import numpy as np
import ml_dtypes

import concourse.bass as bass
import concourse.bacc as bacc
import concourse.mybir as mybir
import concourse.tile as tile
from concourse.bass_utils import run_bass_kernel_spmd

F32 = mybir.dt.float32
F32R = mybir.dt.float32r
BF16 = mybir.dt.bfloat16
AF = mybir.ActivationFunctionType

H, QL, KL, ROPE, NOPE, VD, HID = 128, 1536, 512, 64, 128, 128, 5120
QHD = NOPE + ROPE  # 192
QLEN, KVLEN = 512, 2048
NCORES = 8
HPC = H // NCORES          # 16 heads per core
PAIRS = HPC // 2           # 8 pairs per core
QLC = QL // NCORES         # 192 q-lora rows per core
TSH = QLEN // NCORES       # 64 token rows per output shard
NKC = KVLEN // 128         # 16 kv chunks
NJC = QL // 128            # 12 q-lora chunks
NCC = KL // 128            # 4 compressed-kv chunks
NHID = HID // 128          # 40 hidden chunks
NDS = HID // 512           # 10 output dim slices
EPS = 1e-6





def _host_prepare(inputs):
    """Full inputs -> (list of per-core input dicts, const arrays)."""
    hsq = np.asarray(inputs["hidden_states_q"], np.float32)[0]      # [512, 5120]
    pos = np.asarray(inputs["q_position_ids"])[0]                   # [512]
    ckv_full = np.asarray(inputs["compressed_kv"], np.float32)[0]   # [2048, 576]
    Wqa = np.asarray(inputs["Wqa"], np.float32)                     # [1536, 5120]
    w_ln = np.asarray(inputs["qa_ln_w"], np.float32)                # [1536]
    Wqb = np.asarray(inputs["Wqb"], np.float32)                     # [24576, 1536]
    Wkvb = np.asarray(inputs["Wkvb"], np.float32)                   # [32768, 512]
    Wo = np.asarray(inputs["Wo"], np.float32)                       # [5120, 16384]

    hsqT = np.ascontiguousarray(hsq.T)                              # [5120, 512]
    ckvT = np.ascontiguousarray(ckv_full.T)                         # [576, 2048]
    kpeT = ckvT[KL:]                                                # [64, 2048]
    # c chunks + k_pe duplicated twice (so both heads of a pair can use
    # partition-aligned lhsT slices at base 0 / 64)
    ckv5 = np.concatenate([ckvT[:KL], kpeT, kpeT], axis=0)          # [640, 2048]

    Wqb_w = Wqb * w_ln[None, :]
    qb3 = Wqb_w.reshape(H, QHD, QL)
    kvb = Wkvb.reshape(H, NOPE + VD, KL)
    perm = np.concatenate([np.arange(0, ROPE, 2), np.arange(1, ROPE, 2)])

    # rope tables in half-split layout, [d, t]; doubled over the pair axis
    inv_freq = 1.0 / (10000.0 ** (np.arange(0, ROPE, 2, dtype=np.float64) / ROPE))
    fr = np.outer(pos.astype(np.float64), inv_freq)                 # [512, 32]
    emb = np.concatenate([fr, fr], axis=-1)                         # [512, 64]
    cosT = np.cos(emb).T.astype(np.float32)                         # [64, 512]
    sinT = np.sin(emb).T.astype(np.float32)
    cos2 = np.ascontiguousarray(np.concatenate([cosT, cosT], axis=0))  # [128, 512]
    sin2 = np.ascontiguousarray(np.concatenate([sinT, sinT], axis=0))

    # rot = blockdiag(P, P) @ q'   with  rot_h = [-q'[32:], q'[:32]]
    P64 = np.zeros((ROPE, ROPE), np.float32)
    P64[np.arange(32), np.arange(32) + 32] = -1.0
    P64[np.arange(32, 64), np.arange(32)] = 1.0
    psign = np.zeros((128, 128), np.float32)
    psign[:64, :64] = P64
    psign[64:, 64:] = P64
    psignT = np.ascontiguousarray(psign.T)

    consts = {
        "cos2": cos2,
        "sin2": sin2,
        "psignT": psignT,
        "onesf": np.ones((128, 128), np.float32),
        "onesb": np.ones((128, 1), ml_dtypes.bfloat16),
    }

    in_maps = []
    for c in range(NCORES):
        h0 = c * HPC
        wqaT = np.ascontiguousarray(Wqa[c * QLC:(c + 1) * QLC].T)   # [5120, 192]
        qbt = np.empty((PAIRS, QL, 384), np.float32)
        oabT = np.empty((PAIRS, KL, 2 * VD), np.float32)
        for p in range(PAIRS):
            ha, hb = h0 + 2 * p, h0 + 2 * p + 1
            qbt[p, :, 0:128] = qb3[ha, :NOPE].T
            qbt[p, :, 128:256] = qb3[hb, :NOPE].T
            qbt[p, :, 256:320] = qb3[ha, NOPE:][perm].T
            qbt[p, :, 320:384] = qb3[hb, NOPE:][perm].T
            oabT[p, :, 0:VD] = kvb[ha, NOPE:].T
            oabT[p, :, VD:] = kvb[hb, NOPE:].T
        qab = np.ascontiguousarray(kvb[h0:h0 + HPC, :NOPE, :])      # [16, 128, 512]
        woT = np.ascontiguousarray(
            Wo[:, h0 * VD:(h0 + HPC) * VD].T
        ).astype(ml_dtypes.bfloat16)                                # [2048, 5120]
        in_maps.append({
            "hsqT": hsqT,
            "wqaT": wqaT,
            "qbt": qbt,
            "qab": qab,
            "oabT": oabT,
            "ckv5": ckv5,
            "woT": woT,
        })
    return in_maps, consts


def _build_program(consts):
    nc = bacc.Bacc("TRN2", num_devices=NCORES)

    hsqT = nc.dram_tensor("hsqT", [HID, QLEN], F32R, kind="ExternalInput")
    wqaT = nc.dram_tensor("wqaT", [HID, QLC], F32R, kind="ExternalInput")
    qbt = nc.dram_tensor("qbt", [PAIRS, QL, 384], F32R, kind="ExternalInput")
    qab = nc.dram_tensor("qab", [HPC, NOPE, KL], F32R, kind="ExternalInput")
    oabT = nc.dram_tensor("oabT", [PAIRS, KL, 2 * VD], F32R, kind="ExternalInput")
    ckv5 = nc.dram_tensor("ckv5", [640, KVLEN], F32R, kind="ExternalInput")
    woT = nc.dram_tensor("woT", [HPC * VD, HID], BF16, kind="ExternalInput")
    out_sh = nc.dram_tensor("out_shard", [TSH, HID], F32, kind="ExternalOutput")

    cos2_d = nc.inline_tensor(consts["cos2"], "cos2")
    sin2_d = nc.inline_tensor(consts["sin2"], "sin2")
    psignT_d = nc.inline_tensor(consts["psignT"], "psignT")
    onesf_d = nc.inline_tensor(consts["onesf"], "onesf")
    onesb_d = nc.inline_tensor(consts["onesb"], "onesb")

    # collective bounce buffers (internal DRAM)
    ag_in = nc.dram_tensor("ag_in", [QLC, QLEN], F32)
    ag_out = nc.dram_tensor("ag_out", [QL, QLEN], F32, addr_space="Shared")
    rs_in = nc.dram_tensor("rs_in", [QLEN, HID], F32)
    rs_out = nc.dram_tensor("rs_out", [TSH, HID], F32)
    RG = [list(range(NCORES))]

    with tile.TileContext(nc, num_cores=NCORES) as tc:
        with (
            tc.tile_pool(name="const", bufs=1) as constp,
            tc.tile_pool(name="ckv", bufs=1) as ckvp,
            tc.tile_pool(name="qts", bufs=1) as qtsp,
            tc.tile_pool(name="o16", bufs=1) as o16p,
            tc.tile_pool(name="vdec", bufs=2) as vp,
            tc.tile_pool(name="oab", bufs=2) as oabp,
            tc.tile_pool(name="psV", bufs=1, space="PSUM") as psV,
        ):
            cos2_s = constp.tile([128, QLEN], F32, tag="cos2")
            sin2_s = constp.tile([128, QLEN], F32, tag="sin2")
            psign_s = constp.tile([128, 128], F32, tag="psign")
            onesf_s = constp.tile([128, 128], F32, tag="onesf")
            onesb_s = constp.tile([128, 1], BF16, tag="onesb")
            nc.sync.dma_start(cos2_s[:], cos2_d[:, :])
            nc.sync.dma_start(sin2_s[:], sin2_d[:, :])
            nc.sync.dma_start(psign_s[:], psignT_d[:, :])
            nc.sync.dma_start(onesf_s[:], onesf_d[:, :])
            nc.sync.dma_start(onesb_s[:], onesb_d[:, :])
            psign_r = constp.tile([128, 128], F32R, tag="psign_r")
            onesf_r = constp.tile([128, 128], F32R, tag="onesf_r")
            nc.vector.tensor_copy(psign_r[:], psign_s[:])
            nc.vector.tensor_copy(onesf_r[:], onesf_s[:])

            ckv_s = ckvp.tile([128, 5, KVLEN], F32R)  # 4 c-chunks + [kpe;kpe]
            nc.sync.dma_start(
                ckv_s[:], ckv5[:, :].rearrange("(c p) k -> p c k", p=128)
            )

            qts = qtsp.tile([128, NJC, QLEN], F32R)       # scaled q.T
            o16 = o16p.tile([128, HPC, QLEN], BF16)      # normalized per-head o.T

            def v_decomp(p):
                """Decompress V for pair p -> v tile [128k, kc, 2*VD] bf16."""
                oab_s = oabp.tile([128, NCC, 2 * VD], F32R, tag="oab")
                nc.sync.dma_start(
                    oab_s[:], oabT[p].rearrange("(c p) f -> p c f", p=128)
                )
                v_s = vp.tile([128, NKC, 2 * VD], BF16, tag="v")
                for kc in range(NKC):
                    vps = psV.tile([128, 2 * VD], F32, tag="vps")
                    for ci in range(NCC):
                        nc.tensor.matmul(
                            vps[:],
                            ckv_s[:, ci, kc * 128:(kc + 1) * 128],
                            oab_s[:, ci, :],
                            start=(ci == 0), stop=(ci == NCC - 1),
                        )
                    nc.vector.tensor_copy(v_s[:, kc, :], vps[:])
                return v_s

            # ---------------- stage A: q = hsq @ Wqa.T (sharded) -------------
            with (
                tc.tile_pool(name="stgA", bufs=3) as sap,
                tc.tile_pool(name="psA", bufs=1, space="PSUM") as psA,
            ):
                qra = psA.tile([128, QLEN], F32, tag="qra")
                qrb = psA.tile([64, QLEN], F32, tag="qrb")
                for kc in range(NHID):
                    ht = sap.tile([128, QLEN], F32R, tag="ht")
                    nc.sync.dma_start(ht[:], hsqT[kc * 128:(kc + 1) * 128, :])
                    wt = sap.tile([128, QLC], F32R, tag="wt")
                    nc.sync.dma_start(wt[:], wqaT[kc * 128:(kc + 1) * 128, :])
                    nc.tensor.matmul(
                        qra[:], wt[:, 0:128], ht[:],
                        start=(kc == 0), stop=(kc == NHID - 1),
                    )
                    nc.tensor.matmul(
                        qrb[:], wt[:, 128:QLC], ht[:],
                        start=(kc == 0), stop=(kc == NHID - 1),
                    )
                sqa = sap.tile([128, QLEN], F32, tag="sqa")
                sqb = sap.tile([64, QLEN], F32, tag="sqb")
                nc.scalar.copy(sqa[:], qra[:])
                nc.scalar.copy(sqb[:], qrb[:])
                nc.sync.dma_start(ag_in[0:128, :], sqa[:])
                nc.sync.dma_start(ag_in[128:QLC, :], sqb[:])

            nc.gpsimd.collective_compute(
                "AllGather", mybir.AluOpType.bypass,
                replica_groups=RG,
                ins=[ag_in[:, :].opt()], outs=[ag_out[:, :].opt()],
            )

            # v for pairs 0/1 fills the AllGather wait
            v_tiles = {0: v_decomp(0), 1: v_decomp(1)}

            # -------- stage B: rms scale fold:  qts = qraw * bcast(r) --------
            with (
                tc.tile_pool(name="stgB", bufs=2) as sbp,
                tc.tile_pool(name="stgBs", bufs=1) as sbs,
                tc.tile_pool(name="psB", bufs=1, space="PSUM") as psB,
            ):
                qraw = sbs.tile([128, NJC, QLEN], F32, tag="qraw")
                nc.sync.dma_start(
                    qraw[:], ag_out[:, :].rearrange("(c p) t -> p c t", p=128)
                )
                ssq = psB.tile([1, QLEN], F32, tag="ssq")
                for jc in range(NJC):
                    sq = sbp.tile([128, QLEN], F32R, tag="sq")
                    nc.vector.tensor_mul(sq[:], qraw[:, jc, :], qraw[:, jc, :])
                    nc.tensor.matmul(
                        ssq[:], onesf_r[:, 0:1], sq[:],
                        start=(jc == 0), stop=(jc == NJC - 1),
                    )
                # r_scaled = (1/sqrt(192)) * rsqrt(ssq/1536 + eps)
                #          = 1 / sqrt(ssq*0.125 + 192*eps)
                eps_s = sbs.tile([1, 1], F32, tag="eps")
                nc.gpsimd.memset(eps_s[:], QHD * EPS)
                sqv = sbs.tile([1, QLEN], F32, tag="sqv")
                nc.scalar.activation(
                    sqv[:], ssq[:], AF.Sqrt, scale=QHD / QL, bias=eps_s[:]
                )
                rsc = sbs.tile([1, QLEN], F32R, tag="rsc")
                with nc.allow_low_precision(reason="fp32r"):
                    nc.vector.reciprocal(rsc[:], sqv[:])
                bc = psB.tile([128, QLEN], F32, tag="bc")
                nc.tensor.matmul(
                    bc[:], onesf_r[0:1, :], rsc[:],
                    start=True, stop=True,
                )
                bcs = sbs.tile([128, QLEN], F32, tag="bcs")
                nc.scalar.copy(bcs[:], bc[:])
                for jc in range(NJC):
                    nc.vector.tensor_mul(qts[:, jc, :], qraw[:, jc, :], bcs[:])

            # ---------------- per-pair attention ----------------
            with (
                tc.tile_pool(name="qbt", bufs=2) as qbtp,
                tc.tile_pool(name="qab", bufs=2) as qabp,
                tc.tile_pool(name="hsb", bufs=2) as hsb,     # per-head sbuf
                tc.tile_pool(name="qnp", bufs=2) as qnp,     # qnope sbuf
                tc.tile_pool(name="expp", bufs=3) as expp,
                tc.tile_pool(name="sml", bufs=2) as sml,
                tc.tile_pool(name="psP", bufs=2, space="PSUM") as psP,
                tc.tile_pool(name="psL", bufs=3, space="PSUM") as psL,
                tc.tile_pool(name="psO", bufs=1, space="PSUM") as psO,
                tc.tile_pool(name="psS", bufs=1, space="PSUM") as psS,
            ):
                def load_qbt(p):
                    t = qbtp.tile([128, NJC, 384], F32R, tag="qbt")
                    nc.sync.dma_start(
                        t[:], qbt[p].rearrange("(c p) f -> p c f", p=128)
                    )
                    return t

                def load_qab(h):
                    t = qabp.tile([128, KL], F32R, tag="qab")
                    nc.sync.dma_start(t[:], qab[h])
                    return t

                def pair_qproj(qbt_s):
                    """-> (qn_sb[2], roped) for the pair."""
                    qn_sb = []
                    for part in range(2):   # nope head a, nope head b
                        qn_ps = psP.tile([128, QLEN], F32, tag="pp")
                        for jc in range(NJC):
                            nc.tensor.matmul(
                                qn_ps[:],
                                qbt_s[:, jc, 128 * part:128 * (part + 1)],
                                qts[:, jc, :],
                                start=(jc == 0), stop=(jc == NJC - 1),
                            )
                        s = hsb.tile([128, QLEN], F32R, tag="qn")
                        nc.scalar.copy(s[:], qn_ps[:])
                        qn_sb.append(s)
                    pe_ps = psP.tile([128, QLEN], F32, tag="pp")
                    for jc in range(NJC):
                        nc.tensor.matmul(
                            pe_ps[:], qbt_s[:, jc, 256:384],
                            qts[:, jc, :],
                            start=(jc == 0), stop=(jc == NJC - 1),
                        )
                    pe_sb = hsb.tile([128, QLEN], F32R, tag="pe")
                    nc.scalar.copy(pe_sb[:], pe_ps[:])
                    rot_ps = psP.tile([128, QLEN], F32, tag="pp")
                    nc.tensor.matmul(
                        rot_ps[:], psign_r[:], pe_sb[:],
                        start=True, stop=True,
                    )
                    tmp1 = hsb.tile([128, QLEN], F32, tag="tmp1")
                    nc.vector.tensor_mul(tmp1[:], pe_sb[:], cos2_s[:])
                    tmp2 = hsb.tile([128, QLEN], F32, tag="tmp2")
                    nc.vector.tensor_mul(tmp2[:], rot_ps[:], sin2_s[:])
                    roped = hsb.tile([128, QLEN], F32R, tag="roped")
                    nc.vector.tensor_add(roped[:], tmp1[:], tmp2[:])
                    return qn_sb, roped

                def head_attn(h, hh, qab_s, qn_s, roped, v_s):
                    # q_nope.T = q_absorb.T-chunks @ qn.T
                    qnope_s = qnp.tile([128, NCC, QLEN], F32R, tag="qnope")
                    for ci in range(NCC):
                        qq = psP.tile([128, QLEN], F32, tag="pp")
                        nc.tensor.matmul(
                            qq[:],
                            qab_s[:, ci * 128:(ci + 1) * 128],
                            qn_s[:],
                            start=True, stop=True,
                        )
                        nc.vector.tensor_copy(qnope_s[:, ci, :], qq[:])

                    oT = psO.tile([128, QLEN], F32, tag="ot")
                    ssum = psS.tile([1, QLEN], F32, tag="ssum")
                    for kc in range(NKC):
                        lg = psL.tile([128, QLEN], F32, tag="lg")
                        for ci in range(NCC):
                            nc.tensor.matmul(
                                lg[:],
                                ckv_s[:, ci, kc * 128:(kc + 1) * 128],
                                qnope_s[:, ci, :],
                                start=(ci == 0), stop=False,
                            )
                        nc.tensor.matmul(
                            lg[:],
                            ckv_s[:, 4, kc * 128:(kc + 1) * 128][
                                64 * hh:64 * (hh + 1), :],
                            roped[64 * hh:64 * (hh + 1), :],
                            start=False, stop=True,
                        )
                        ex = expp.tile([128, QLEN], BF16, tag="ex")
                        nc.scalar.activation(ex[:], lg[:], AF.Exp)
                        nc.tensor.matmul(
                            oT[:],
                            v_s[:, kc, VD * hh:VD * (hh + 1)],
                            ex[:],
                            start=(kc == 0), stop=(kc == NKC - 1),
                        )
                        nc.tensor.matmul(
                            ssum[:], onesb_s[:], ex[:],
                            start=(kc == 0), stop=(kc == NKC - 1),
                        )
                    inv = sml.tile([1, QLEN], F32R, tag="inv")
                    with nc.allow_low_precision(reason="fp32r"):
                        nc.vector.reciprocal(inv[:], ssum[:])
                    bcp = psP.tile([128, QLEN], F32, tag="pp")
                    nc.tensor.matmul(
                        bcp[:], onesf_r[0:1, :], inv[:],
                        start=True, stop=True,
                    )
                    binv = sml.tile([128, QLEN], F32, tag="binv")
                    nc.scalar.copy(binv[:], bcp[:])
                    nc.vector.tensor_mul(o16[:, h, :], oT[:], binv[:])

                qbt_tiles = {0: load_qbt(0)}
                qab_tiles = {0: load_qab(0)}
                for p in range(PAIRS):
                    qbt_s = qbt_tiles.pop(p)
                    if p + 1 < PAIRS:
                        qbt_tiles[p + 1] = load_qbt(p + 1)
                    qn_sb, roped = pair_qproj(qbt_s)
                    v_s = v_tiles.pop(p)
                    for hh in range(2):
                        h = 2 * p + hh
                        qab_s = qab_tiles.pop(h)
                        if h + 1 < HPC:
                            qab_tiles[h + 1] = load_qab(h + 1)
                        head_attn(h, hh, qab_s, qn_sb[hh], roped, v_s)
                    if p + 2 < PAIRS:
                        v_tiles[p + 2] = v_decomp(p + 2)

            # ---------------- output projection + ReduceScatter --------------
            with (
                tc.tile_pool(name="wo", bufs=2) as wop,
                tc.tile_pool(name="osb", bufs=3) as osb,
                tc.tile_pool(name="psW", bufs=2, space="PSUM") as psW,
            ):
                for ds in range(NDS):
                    w16 = wop.tile([128, HPC, 512], BF16, tag="w16")
                    nc.sync.dma_start(
                        w16[:],
                        woT[:, :].rearrange("(g p) d -> p g d", p=128)[
                            :, :, ds * 512:(ds + 1) * 512],
                    )
                    for tc4 in range(4):
                        acc = psW.tile([128, 512], F32, tag="acc")
                        for g in range(HPC):
                            nc.tensor.matmul(
                                acc[:],
                                o16[:, g, tc4 * 128:(tc4 + 1) * 128],
                                w16[:, g, :],
                                start=(g == 0), stop=(g == HPC - 1),
                            )
                        ot = osb.tile([128, 512], F32, tag="ot")
                        nc.vector.tensor_copy(ot[:], acc[:])
                        nc.sync.dma_start(
                            rs_in[tc4 * 128:(tc4 + 1) * 128,
                                  ds * 512:(ds + 1) * 512],
                            ot[:],
                        )

            nc.gpsimd.collective_compute(
                "ReduceScatter", mybir.AluOpType.add,
                replica_groups=RG,
                ins=[rs_in[:, :].opt()], outs=[rs_out[:, :].opt()],
            )
            nc.gpsimd.dma_start(out_sh[:, :], rs_out[:, :])

    nc.compile()
    return nc


_CACHE = {}


def _get_program(consts):
    key = (consts["cos2"].tobytes(), consts["sin2"].tobytes())
    if key not in _CACHE:
        _CACHE[key] = _build_program(consts)
    return _CACHE[key]


def _run(inputs, **kwargs):
    in_maps, consts = _host_prepare(inputs)
    nc = _get_program(consts)
    res = run_bass_kernel_spmd(nc, in_maps, core_ids=list(range(NCORES)), **kwargs)
    shards = [res.results[c]["out_shard"] for c in range(NCORES)]
    out = np.concatenate(shards, axis=0)[None].astype(np.float32)
    return out, res


def kernel(**inputs) -> np.ndarray:
    return _run(inputs)[0]


# revision 9
# speedup vs baseline: 1.0975x; 1.0975x over previous
"""DeepSeek-V2 MLA attention (weight-absorbed) on 8 Trainium2 NeuronCores.

Sharding: tensor-parallel over the 128 heads (16 heads/core).  The q
LoRA projection (hidden @ Wqa.T) is sharded over the Q_LORA output dim
and AllGathered; the per-head attention runs fully local; the output
projection partials are summed with a ReduceScatter over the token axis
and the 8 shards are concatenated on the host.

Math restructuring vs the reference (exactly associativity-equivalent):
  - q_nope = (q @ qb_nope.T) @ q_absorb          (factor through the 128-dim)
  - o      = softmax(l) @ (ckv @ out_absorb.T)   (decompress V, 128-dim)
  - out    = concat_h(o_h) @ Wo.T                (plain o_proj)
  - rmsnorm's per-token scale and 1/sqrt(192) are folded into q;
    qa_ln_w is folded into Wqb; the RoPE interleave permutation is
    folded into the rope rows of Wqb; softmax skips the max-subtraction
    (logits are O(3) for this problem) and normalizes o after PV.
"""

import math
import numpy as np
import ml_dtypes

import concourse.bass as bass
import concourse.bacc as bacc
import concourse.mybir as mybir
import concourse.tile as tile
from concourse.bass_utils import run_bass_kernel_spmd

F32 = mybir.dt.float32
F32R = mybir.dt.float32r
BF16 = mybir.dt.bfloat16
AF = mybir.ActivationFunctionType

H, QL, KL, ROPE, NOPE, VD, HID = 128, 1536, 512, 64, 128, 128, 5120
QHD = NOPE + ROPE  # 192
QLEN, KVLEN = 512, 2048
NCORES = 8
HPC = H // NCORES          # 16 heads per core
PAIRS = HPC // 2           # 8 pairs per core
QLC = QL // NCORES         # 192 q-lora rows per core
TSH = QLEN // NCORES       # 64 token rows per output shard
NKC = KVLEN // 128         # 16 kv chunks
NJC = QL // 128            # 12 q-lora chunks
NCC = KL // 128            # 4 compressed-kv chunks
NHID = HID // 128          # 40 hidden chunks
NDS = HID // 512           # 10 output dim slices
EPS = 1e-6





def _host_prepare(inputs):
    """Full inputs -> (list of per-core input dicts, const arrays)."""
    hsq = np.asarray(inputs["hidden_states_q"], np.float32)[0]      # [512, 5120]
    pos = np.asarray(inputs["q_position_ids"])[0]                   # [512]
    ckv_full = np.asarray(inputs["compressed_kv"], np.float32)[0]   # [2048, 576]
    Wqa = np.asarray(inputs["Wqa"], np.float32)                     # [1536, 5120]
    w_ln = np.asarray(inputs["qa_ln_w"], np.float32)                # [1536]
    Wqb = np.asarray(inputs["Wqb"], np.float32)                     # [24576, 1536]
    Wkvb = np.asarray(inputs["Wkvb"], np.float32)                   # [32768, 512]
    Wo = np.asarray(inputs["Wo"], np.float32)                       # [5120, 16384]

    hsqT = np.ascontiguousarray(hsq.T)                              # [5120, 512]
    ckvT = np.ascontiguousarray(ckv_full.T)                         # [576, 2048]
    kpeT = ckvT[KL:]                                                # [64, 2048]
    # c chunks + k_pe duplicated twice (so both heads of a pair can use
    # partition-aligned lhsT slices at base 0 / 64)
    ckv5 = np.concatenate([ckvT[:KL], kpeT, kpeT], axis=0).astype(
        ml_dtypes.bfloat16)                                         # [640, 2048]

    Wqb_w = Wqb * w_ln[None, :]
    qb3 = Wqb_w.reshape(H, QHD, QL)
    kvb = Wkvb.reshape(H, NOPE + VD, KL)
    perm = np.concatenate([np.arange(0, ROPE, 2), np.arange(1, ROPE, 2)])

    # rope tables in half-split layout, [d, t]; doubled over the pair axis
    inv_freq = 1.0 / (10000.0 ** (np.arange(0, ROPE, 2, dtype=np.float64) / ROPE))
    fr = np.outer(pos.astype(np.float64), inv_freq)                 # [512, 32]
    emb = np.concatenate([fr, fr], axis=-1)                         # [512, 64]
    cosT = np.cos(emb).T.astype(np.float32)                         # [64, 512]
    sinT = np.sin(emb).T.astype(np.float32)
    cos2 = np.ascontiguousarray(np.concatenate([cosT, cosT], axis=0))  # [128, 512]
    sin2 = np.ascontiguousarray(np.concatenate([sinT, sinT], axis=0))

    # rot = blockdiag(P, P) @ q'   with  rot_h = [-q'[32:], q'[:32]]
    P64 = np.zeros((ROPE, ROPE), np.float32)
    P64[np.arange(32), np.arange(32) + 32] = -1.0
    P64[np.arange(32, 64), np.arange(32)] = 1.0
    psign = np.zeros((128, 128), np.float32)
    psign[:64, :64] = P64
    psign[64:, 64:] = P64
    psignT = np.ascontiguousarray(psign.T)

    consts = {
        "cos2": cos2,
        "sin2": sin2,
        "psignT": psignT.astype(ml_dtypes.bfloat16),
        "onesf": np.ones((128, 128), np.float32),
        "onesb": np.ones((128, 1), ml_dtypes.bfloat16),
    }

    in_maps = []
    for c in range(NCORES):
        h0 = c * HPC
        wqaT = np.ascontiguousarray(Wqa[c * QLC:(c + 1) * QLC].T)   # [5120, 192]
        qbt = np.empty((PAIRS, QL, 384), ml_dtypes.bfloat16)
        oabT = np.empty((PAIRS, KL, 2 * VD), ml_dtypes.bfloat16)
        for p in range(PAIRS):
            ha, hb = h0 + 2 * p, h0 + 2 * p + 1
            qbt[p, :, 0:128] = qb3[ha, :NOPE].T
            qbt[p, :, 128:256] = qb3[hb, :NOPE].T
            qbt[p, :, 256:320] = qb3[ha, NOPE:][perm].T
            qbt[p, :, 320:384] = qb3[hb, NOPE:][perm].T
            oabT[p, :, 0:VD] = kvb[ha, NOPE:].T
            oabT[p, :, VD:] = kvb[hb, NOPE:].T
        qab = np.ascontiguousarray(kvb[h0:h0 + HPC, :NOPE, :]).astype(
            ml_dtypes.bfloat16)                                         # [16, 128, 512]
        woT = np.ascontiguousarray(
            Wo[:, h0 * VD:(h0 + HPC) * VD].T
        ).astype(ml_dtypes.bfloat16)                                # [2048, 5120]
        in_maps.append({
            "hsqT": hsqT,
            "wqaT": wqaT,
            "qbt": qbt,
            "qab": qab,
            "oabT": oabT,
            "ckv5": ckv5,
            "woT": woT,
        })
    return in_maps, consts


def _build_program(consts):
    nc = bacc.Bacc("TRN2", num_devices=NCORES)

    hsqT = nc.dram_tensor("hsqT", [HID, QLEN], F32R, kind="ExternalInput")
    wqaT = nc.dram_tensor("wqaT", [HID, QLC], F32R, kind="ExternalInput")
    qbt = nc.dram_tensor("qbt", [PAIRS, QL, 384], BF16, kind="ExternalInput")
    qab = nc.dram_tensor("qab", [HPC, NOPE, KL], BF16, kind="ExternalInput")
    oabT = nc.dram_tensor("oabT", [PAIRS, KL, 2 * VD], BF16, kind="ExternalInput")
    ckv5 = nc.dram_tensor("ckv5", [640, KVLEN], BF16, kind="ExternalInput")
    woT = nc.dram_tensor("woT", [HPC * VD, HID], BF16, kind="ExternalInput")
    out_sh = nc.dram_tensor("out_shard", [TSH, HID], F32, kind="ExternalOutput")

    cos2_d = nc.inline_tensor(consts["cos2"], "cos2")
    sin2_d = nc.inline_tensor(consts["sin2"], "sin2")
    psignT_d = nc.inline_tensor(consts["psignT"], "psignT")
    onesf_d = nc.inline_tensor(consts["onesf"], "onesf")
    onesb_d = nc.inline_tensor(consts["onesb"], "onesb")

    # collective bounce buffers (internal DRAM)
    ag_in = nc.dram_tensor("ag_in", [QLC, QLEN], F32)
    ag_out = nc.dram_tensor("ag_out", [QL, QLEN], F32, addr_space="Shared")
    rs_in = nc.dram_tensor("rs_in", [QLEN, HID], F32)
    rs_out = nc.dram_tensor("rs_out", [TSH, HID], F32)
    RG = [list(range(NCORES))]

    with tile.TileContext(nc, num_cores=NCORES) as tc:
        with (
            tc.tile_pool(name="const", bufs=1) as constp,
            tc.tile_pool(name="ckv", bufs=1) as ckvp,
            tc.tile_pool(name="qts", bufs=1) as qtsp,
            tc.tile_pool(name="o16", bufs=1) as o16p,
            tc.tile_pool(name="vdec", bufs=2) as vp,
            tc.tile_pool(name="oab", bufs=2) as oabp,
            tc.tile_pool(name="psV", bufs=1, space="PSUM") as psV,
        ):
            cos2_s = constp.tile([128, QLEN], F32, tag="cos2")
            sin2_s = constp.tile([128, QLEN], F32, tag="sin2")
            psign_s = constp.tile([128, 128], BF16, tag="psign")
            onesf_s = constp.tile([128, 128], F32, tag="onesf")
            onesb_s = constp.tile([128, 1], BF16, tag="onesb")
            nc.sync.dma_start(cos2_s[:], cos2_d[:, :])
            nc.sync.dma_start(sin2_s[:], sin2_d[:, :])
            nc.sync.dma_start(psign_s[:], psignT_d[:, :])
            nc.sync.dma_start(onesf_s[:], onesf_d[:, :])
            nc.sync.dma_start(onesb_s[:], onesb_d[:, :])
            onesf_r = constp.tile([128, 128], F32R, tag="onesf_r")
            nc.vector.tensor_copy(onesf_r[:], onesf_s[:])

            ckv_s = ckvp.tile([128, 5, KVLEN], BF16)  # 4 c-chunks + [kpe;kpe]
            nc.sync.dma_start(
                ckv_s[:], ckv5[:, :].rearrange("(c p) k -> p c k", p=128)
            )

            qts = qtsp.tile([128, NJC, QLEN], BF16)       # scaled q.T
            o16 = o16p.tile([128, HPC, QLEN], BF16)      # normalized per-head o.T

            def v_decomp(p):
                """Decompress V for pair p -> v tile [128k, kc, 2*VD] bf16."""
                oab_s = oabp.tile([128, NCC, 2 * VD], BF16, tag="oab")
                nc.sync.dma_start(
                    oab_s[:], oabT[p].rearrange("(c p) f -> p c f", p=128)
                )
                v_s = vp.tile([128, NKC, 2 * VD], BF16, tag="v")
                for kc in range(NKC):
                    vps = psV.tile([128, 2 * VD], F32, tag="vps")
                    for ci in range(NCC):
                        nc.tensor.matmul(
                            vps[:],
                            ckv_s[:, ci, kc * 128:(kc + 1) * 128],
                            oab_s[:, ci, :],
                            start=(ci == 0), stop=(ci == NCC - 1),
                        )
                    nc.vector.tensor_copy(v_s[:, kc, :], vps[:])
                return v_s

            # ---------------- stage A: q = hsq @ Wqa.T (sharded) -------------
            with (
                tc.tile_pool(name="stgA", bufs=3) as sap,
                tc.tile_pool(name="psA", bufs=1, space="PSUM") as psA,
            ):
                qra = psA.tile([128, QLEN], F32, tag="qra")
                qrb = psA.tile([64, QLEN], F32, tag="qrb")
                for kc in range(NHID):
                    ht = sap.tile([128, QLEN], F32R, tag="ht")
                    nc.sync.dma_start(ht[:], hsqT[kc * 128:(kc + 1) * 128, :])
                    wt = sap.tile([128, QLC], F32R, tag="wt")
                    nc.sync.dma_start(wt[:], wqaT[kc * 128:(kc + 1) * 128, :])
                    nc.tensor.matmul(
                        qra[:], wt[:, 0:128], ht[:],
                        start=(kc == 0), stop=(kc == NHID - 1),
                    )
                    nc.tensor.matmul(
                        qrb[:], wt[:, 128:QLC], ht[:],
                        start=(kc == 0), stop=(kc == NHID - 1),
                    )
                sqa = sap.tile([128, QLEN], F32, tag="sqa")
                sqb = sap.tile([64, QLEN], F32, tag="sqb")
                nc.scalar.copy(sqa[:], qra[:])
                nc.scalar.copy(sqb[:], qrb[:])
                nc.sync.dma_start(ag_in[0:128, :], sqa[:])
                nc.sync.dma_start(ag_in[128:QLC, :], sqb[:])

            nc.gpsimd.collective_compute(
                "AllGather", mybir.AluOpType.bypass,
                replica_groups=RG,
                ins=[ag_in[:, :].opt()], outs=[ag_out[:, :].opt()],
            )

            # v for pairs 0/1 fills the AllGather wait
            v_tiles = {0: v_decomp(0), 1: v_decomp(1)}

            # -------- stage B: rms scale fold:  qts = qraw * bcast(r) --------
            with (
                tc.tile_pool(name="stgB", bufs=2) as sbp,
                tc.tile_pool(name="stgBs", bufs=1) as sbs,
                tc.tile_pool(name="psB", bufs=1, space="PSUM") as psB,
            ):
                qraw = sbs.tile([128, NJC, QLEN], F32, tag="qraw")
                nc.sync.dma_start(
                    qraw[:], ag_out[:, :].rearrange("(c p) t -> p c t", p=128)
                )
                ssq = psB.tile([1, QLEN], F32, tag="ssq")
                for jc in range(NJC):
                    sq = sbp.tile([128, QLEN], F32R, tag="sq")
                    nc.vector.tensor_mul(sq[:], qraw[:, jc, :], qraw[:, jc, :])
                    nc.tensor.matmul(
                        ssq[:], onesf_r[:, 0:1], sq[:],
                        start=(jc == 0), stop=(jc == NJC - 1),
                    )
                # r_scaled = (1/sqrt(192)) * rsqrt(ssq/1536 + eps)
                #          = 1 / sqrt(ssq*0.125 + 192*eps)
                eps_s = sbs.tile([1, 1], F32, tag="eps")
                nc.gpsimd.memset(eps_s[:], QHD * EPS)
                sqv = sbs.tile([1, QLEN], F32, tag="sqv")
                nc.scalar.activation(
                    sqv[:], ssq[:], AF.Sqrt, scale=QHD / QL, bias=eps_s[:]
                )
                rsc = sbs.tile([1, QLEN], F32R, tag="rsc")
                with nc.allow_low_precision(reason="fp32r"):
                    nc.vector.reciprocal(rsc[:], sqv[:])
                bc = psB.tile([128, QLEN], F32, tag="bc")
                nc.tensor.matmul(
                    bc[:], onesf_r[0:1, :], rsc[:],
                    start=True, stop=True,
                )
                bcs = sbs.tile([128, QLEN], F32, tag="bcs")
                nc.scalar.copy(bcs[:], bc[:])
                for jc in range(NJC):
                    nc.vector.tensor_mul(qts[:, jc, :], qraw[:, jc, :], bcs[:])

            # ---------------- per-pair attention ----------------
            with (
                tc.tile_pool(name="qbt", bufs=2) as qbtp,
                tc.tile_pool(name="qab", bufs=2) as qabp,
                tc.tile_pool(name="hsb", bufs=2) as hsb,     # per-head sbuf
                tc.tile_pool(name="qnp", bufs=2) as qnp,     # qnope sbuf
                tc.tile_pool(name="expp", bufs=3) as expp,
                tc.tile_pool(name="sml", bufs=2) as sml,
                tc.tile_pool(name="psP", bufs=2, space="PSUM") as psP,
                tc.tile_pool(name="psL", bufs=3, space="PSUM") as psL,
                tc.tile_pool(name="psO", bufs=1, space="PSUM") as psO,
                tc.tile_pool(name="psS", bufs=1, space="PSUM") as psS,
            ):
                def load_qbt(p):
                    t = qbtp.tile([128, NJC, 384], BF16, tag="qbt")
                    nc.sync.dma_start(
                        t[:], qbt[p].rearrange("(c p) f -> p c f", p=128)
                    )
                    return t

                def load_qab(h):
                    t = qabp.tile([128, KL], BF16, tag="qab")
                    nc.sync.dma_start(t[:], qab[h])
                    return t

                def pair_qproj(qbt_s):
                    """-> (qn_sb[2], roped) for the pair."""
                    qn_sb = []
                    for part in range(2):   # nope head a, nope head b
                        qn_ps = psP.tile([128, QLEN], F32, tag="pp")
                        for jc in range(NJC):
                            nc.tensor.matmul(
                                qn_ps[:],
                                qbt_s[:, jc, 128 * part:128 * (part + 1)],
                                qts[:, jc, :],
                                start=(jc == 0), stop=(jc == NJC - 1),
                            )
                        s = hsb.tile([128, QLEN], BF16, tag="qn")
                        nc.scalar.copy(s[:], qn_ps[:])
                        qn_sb.append(s)
                    pe_ps = psP.tile([128, QLEN], F32, tag="pp")
                    for jc in range(NJC):
                        nc.tensor.matmul(
                            pe_ps[:], qbt_s[:, jc, 256:384],
                            qts[:, jc, :],
                            start=(jc == 0), stop=(jc == NJC - 1),
                        )
                    pe_sb = hsb.tile([128, QLEN], BF16, tag="pe")
                    nc.scalar.copy(pe_sb[:], pe_ps[:])
                    rot_ps = psP.tile([128, QLEN], F32, tag="pp")
                    nc.tensor.matmul(
                        rot_ps[:], psign_s[:], pe_sb[:],
                        start=True, stop=True,
                    )
                    tmp1 = hsb.tile([128, QLEN], F32, tag="tmp1")
                    nc.vector.tensor_mul(tmp1[:], pe_sb[:], cos2_s[:])
                    tmp2 = hsb.tile([128, QLEN], F32, tag="tmp2")
                    nc.vector.tensor_mul(tmp2[:], rot_ps[:], sin2_s[:])
                    roped = hsb.tile([128, QLEN], BF16, tag="roped")
                    nc.vector.tensor_add(roped[:], tmp1[:], tmp2[:])
                    return qn_sb, roped

                def head_attn(h, hh, qab_s, qn_s, roped, v_s):
                    # q_nope.T = q_absorb.T-chunks @ qn.T
                    qnope_s = qnp.tile([128, NCC, QLEN], BF16, tag="qnope")
                    for ci in range(NCC):
                        qq = psP.tile([128, QLEN], F32, tag="pp")
                        nc.tensor.matmul(
                            qq[:],
                            qab_s[:, ci * 128:(ci + 1) * 128],
                            qn_s[:],
                            start=True, stop=True,
                        )
                        nc.vector.tensor_copy(qnope_s[:, ci, :], qq[:])

                    oT = psO.tile([128, QLEN], F32, tag="ot")
                    ssum = psS.tile([1, QLEN], F32, tag="ssum")
                    for kc in range(NKC):
                        lg = psL.tile([128, QLEN], F32, tag="lg")
                        for ci in range(NCC):
                            nc.tensor.matmul(
                                lg[:],
                                ckv_s[:, ci, kc * 128:(kc + 1) * 128],
                                qnope_s[:, ci, :],
                                start=(ci == 0), stop=False,
                            )
                        nc.tensor.matmul(
                            lg[:],
                            ckv_s[:, 4, kc * 128:(kc + 1) * 128][
                                64 * hh:64 * (hh + 1), :],
                            roped[64 * hh:64 * (hh + 1), :],
                            start=False, stop=True,
                        )
                        ex = expp.tile([128, QLEN], BF16, tag="ex")
                        nc.scalar.activation(ex[:], lg[:], AF.Exp)
                        nc.tensor.matmul(
                            oT[:],
                            v_s[:, kc, VD * hh:VD * (hh + 1)],
                            ex[:],
                            start=(kc == 0), stop=(kc == NKC - 1),
                        )
                        nc.tensor.matmul(
                            ssum[:], onesb_s[:], ex[:],
                            start=(kc == 0), stop=(kc == NKC - 1),
                        )
                    inv = sml.tile([1, QLEN], F32R, tag="inv")
                    with nc.allow_low_precision(reason="fp32r"):
                        nc.vector.reciprocal(inv[:], ssum[:])
                    bcp = psP.tile([128, QLEN], F32, tag="pp")
                    nc.tensor.matmul(
                        bcp[:], onesf_r[0:1, :], inv[:],
                        start=True, stop=True,
                    )
                    binv = sml.tile([128, QLEN], F32, tag="binv")
                    nc.scalar.copy(binv[:], bcp[:])
                    nc.vector.tensor_mul(o16[:, h, :], oT[:], binv[:])

                qbt_tiles = {0: load_qbt(0)}
                qab_tiles = {0: load_qab(0)}
                for p in range(PAIRS):
                    qbt_s = qbt_tiles.pop(p)
                    if p + 1 < PAIRS:
                        qbt_tiles[p + 1] = load_qbt(p + 1)
                    qn_sb, roped = pair_qproj(qbt_s)
                    v_s = v_tiles.pop(p)
                    for hh in range(2):
                        h = 2 * p + hh
                        qab_s = qab_tiles.pop(h)
                        if h + 1 < HPC:
                            qab_tiles[h + 1] = load_qab(h + 1)
                        head_attn(h, hh, qab_s, qn_sb[hh], roped, v_s)
                    if p + 2 < PAIRS:
                        v_tiles[p + 2] = v_decomp(p + 2)

            # ---------------- output projection + ReduceScatter --------------
            with (
                tc.tile_pool(name="wo", bufs=2) as wop,
                tc.tile_pool(name="osb", bufs=3) as osb,
                tc.tile_pool(name="psW", bufs=2, space="PSUM") as psW,
            ):
                for ds in range(NDS):
                    w16 = wop.tile([128, HPC, 512], BF16, tag="w16")
                    nc.sync.dma_start(
                        w16[:],
                        woT[:, :].rearrange("(g p) d -> p g d", p=128)[
                            :, :, ds * 512:(ds + 1) * 512],
                    )
                    for tc4 in range(4):
                        acc = psW.tile([128, 512], F32, tag="acc")
                        for g in range(HPC):
                            nc.tensor.matmul(
                                acc[:],
                                o16[:, g, tc4 * 128:(tc4 + 1) * 128],
                                w16[:, g, :],
                                start=(g == 0), stop=(g == HPC - 1),
                            )
                        ot = osb.tile([128, 512], F32, tag="ot")
                        nc.vector.tensor_copy(ot[:], acc[:])
                        nc.sync.dma_start(
                            rs_in[tc4 * 128:(tc4 + 1) * 128,
                                  ds * 512:(ds + 1) * 512],
                            ot[:],
                        )

            nc.gpsimd.collective_compute(
                "ReduceScatter", mybir.AluOpType.add,
                replica_groups=RG,
                ins=[rs_in[:, :].opt()], outs=[rs_out[:, :].opt()],
            )
            nc.gpsimd.dma_start(out_sh[:, :], rs_out[:, :])

    nc.compile()
    return nc


_CACHE = {}


def _get_program(consts):
    key = (consts["cos2"].tobytes(), consts["sin2"].tobytes())
    if key not in _CACHE:
        _CACHE[key] = _build_program(consts)
    return _CACHE[key]


def _run(inputs, **kwargs):
    in_maps, consts = _host_prepare(inputs)
    nc = _get_program(consts)
    res = run_bass_kernel_spmd(nc, in_maps, core_ids=list(range(NCORES)), **kwargs)
    shards = [res.results[c]["out_shard"] for c in range(NCORES)]
    out = np.concatenate(shards, axis=0)[None].astype(np.float32)
    return out, res


def kernel(**inputs) -> np.ndarray:
    return _run(inputs)[0]


# revision 15
# speedup vs baseline: 1.2300x; 1.1208x over previous
"""DeepSeek-V2 MLA attention (weight-absorbed) on 8 Trainium2 NeuronCores.

Sharding: tensor-parallel over the 128 heads (16 heads/core).  The q
LoRA projection (hidden @ Wqa.T) is sharded over the Q_LORA output dim
and AllGathered; the per-head attention runs fully local; the output
projection partials are summed with a ReduceScatter over the token axis
and the 8 shards are concatenated on the host.

Math restructuring vs the reference (exactly associativity-equivalent):
  - q_nope = (q @ qb_nope.T) @ q_absorb          (factor through the 128-dim)
  - o      = softmax(l) @ (ckv @ out_absorb.T)   (decompress V, 128-dim)
  - out    = concat_h(o_h) @ Wo.T                (plain o_proj)
  - rmsnorm's per-token scale and 1/sqrt(192) are folded into q;
    qa_ln_w is folded into Wqb; the RoPE interleave permutation is
    folded into the rope rows of Wqb; softmax skips the max-subtraction
    (logits are O(3) for this problem) and normalizes o after PV.
"""

import math
import numpy as np
import ml_dtypes

import concourse.bass as bass
import concourse.bacc as bacc
import concourse.mybir as mybir
import concourse.tile as tile
from concourse.bass_utils import run_bass_kernel_spmd

F32 = mybir.dt.float32
F32R = mybir.dt.float32r
BF16 = mybir.dt.bfloat16
AF = mybir.ActivationFunctionType

H, QL, KL, ROPE, NOPE, VD, HID = 128, 1536, 512, 64, 128, 128, 5120
QHD = NOPE + ROPE  # 192
QLEN, KVLEN = 512, 2048
NCORES = 8
HPC = H // NCORES          # 16 heads per core
PAIRS = HPC // 2           # 8 pairs per core
QLC = QL // NCORES         # 192 q-lora rows per core
TSH = QLEN // NCORES       # 64 token rows per output shard
NKC = KVLEN // 128         # 16 kv chunks
NJC = QL // 128            # 12 q-lora chunks
NCC = KL // 128            # 4 compressed-kv chunks
NHID = HID // 128          # 40 hidden chunks
NDS = HID // 512           # 10 output dim slices
EPS = 1e-6





def _host_prepare(inputs):
    """Full inputs -> (list of per-core input dicts, const arrays)."""
    hsq = np.asarray(inputs["hidden_states_q"], np.float32)[0]      # [512, 5120]
    pos = np.asarray(inputs["q_position_ids"])[0]                   # [512]
    ckv_full = np.asarray(inputs["compressed_kv"], np.float32)[0]   # [2048, 576]
    Wqa = np.asarray(inputs["Wqa"], np.float32)                     # [1536, 5120]
    w_ln = np.asarray(inputs["qa_ln_w"], np.float32)                # [1536]
    Wqb = np.asarray(inputs["Wqb"], np.float32)                     # [24576, 1536]
    Wkvb = np.asarray(inputs["Wkvb"], np.float32)                   # [32768, 512]
    Wo = np.asarray(inputs["Wo"], np.float32)                       # [5120, 16384]

    hsqT = np.ascontiguousarray(hsq.T)                              # [5120, 512]
    ckvT = np.ascontiguousarray(ckv_full.T)                         # [576, 2048]
    kpeT = ckvT[KL:]                                                # [64, 2048]
    # c chunks + k_pe duplicated twice (so both heads of a pair can use
    # partition-aligned lhsT slices at base 0 / 64)
    ckv5 = np.concatenate([ckvT[:KL], kpeT, kpeT], axis=0).astype(
        ml_dtypes.bfloat16)                                         # [640, 2048]

    Wqb_w = Wqb * w_ln[None, :]
    qb3 = Wqb_w.reshape(H, QHD, QL)
    kvb = Wkvb.reshape(H, NOPE + VD, KL)
    perm = np.concatenate([np.arange(0, ROPE, 2), np.arange(1, ROPE, 2)])

    # rope tables in half-split layout, [d, t]; doubled over the pair axis
    inv_freq = 1.0 / (10000.0 ** (np.arange(0, ROPE, 2, dtype=np.float64) / ROPE))
    fr = np.outer(pos.astype(np.float64), inv_freq)                 # [512, 32]
    emb = np.concatenate([fr, fr], axis=-1)                         # [512, 64]
    cosT = np.cos(emb).T.astype(np.float32)                         # [64, 512]
    sinT = np.sin(emb).T.astype(np.float32)
    cos2 = np.ascontiguousarray(np.concatenate([cosT, cosT], axis=0))  # [128, 512]
    sin2 = np.ascontiguousarray(np.concatenate([sinT, sinT], axis=0))

    # rot = blockdiag(P, P) @ q'   with  rot_h = [-q'[32:], q'[:32]]
    P64 = np.zeros((ROPE, ROPE), np.float32)
    P64[np.arange(32), np.arange(32) + 32] = -1.0
    P64[np.arange(32, 64), np.arange(32)] = 1.0
    psign = np.zeros((128, 128), np.float32)
    psign[:64, :64] = P64
    psign[64:, 64:] = P64
    psignT = np.ascontiguousarray(psign.T)

    consts = {
        "cos2": cos2,
        "sin2": sin2,
        "psignT": psignT.astype(ml_dtypes.bfloat16),
        "onesf": np.ones((128, 128), np.float32),
        "onesb": np.ones((128, 1), ml_dtypes.bfloat16),
    }

    in_maps = []
    for c in range(NCORES):
        h0 = c * HPC
        wqaT = np.ascontiguousarray(Wqa[c * QLC:(c + 1) * QLC].T)   # [5120, 192]
        qbt = np.empty((PAIRS, QL, 384), ml_dtypes.bfloat16)
        abT = np.empty((PAIRS, KL, 4 * VD), ml_dtypes.bfloat16)
        for p in range(PAIRS):
            ha, hb = h0 + 2 * p, h0 + 2 * p + 1
            qbt[p, :, 0:128] = qb3[ha, :NOPE].T
            qbt[p, :, 128:256] = qb3[hb, :NOPE].T
            qbt[p, :, 256:320] = qb3[ha, NOPE:][perm].T
            qbt[p, :, 320:384] = qb3[hb, NOPE:][perm].T
            abT[p, :, 0:128] = kvb[ha, :NOPE].T      # q_absorb.T head a
            abT[p, :, 128:256] = kvb[hb, :NOPE].T    # q_absorb.T head b
            abT[p, :, 256:384] = kvb[ha, NOPE:].T    # out_absorb.T head a
            abT[p, :, 384:512] = kvb[hb, NOPE:].T    # out_absorb.T head b
        woT = np.ascontiguousarray(
            Wo[:, h0 * VD:(h0 + HPC) * VD].T
        ).astype(ml_dtypes.bfloat16)                                # [2048, 5120]
        in_maps.append({
            "hsqT": hsqT,
            "wqaT": wqaT,
            "qbt": qbt,
            "abT": abT,
            "ckv5": ckv5,
            "woT": woT,
        })
    return in_maps, consts


def _build_program(consts):
    nc = bacc.Bacc("TRN2", num_devices=NCORES)

    hsqT = nc.dram_tensor("hsqT", [HID, QLEN], F32R, kind="ExternalInput")
    wqaT = nc.dram_tensor("wqaT", [HID, QLC], F32R, kind="ExternalInput")
    qbt = nc.dram_tensor("qbt", [PAIRS, QL, 384], BF16, kind="ExternalInput")
    abT = nc.dram_tensor("abT", [PAIRS, KL, 4 * VD], BF16, kind="ExternalInput")
    ckv5 = nc.dram_tensor("ckv5", [640, KVLEN], BF16, kind="ExternalInput")
    woT = nc.dram_tensor("woT", [HPC * VD, HID], BF16, kind="ExternalInput")
    out_sh = nc.dram_tensor("out_shard", [TSH, HID], F32, kind="ExternalOutput")

    cos2_d = nc.inline_tensor(consts["cos2"], "cos2")
    sin2_d = nc.inline_tensor(consts["sin2"], "sin2")
    psignT_d = nc.inline_tensor(consts["psignT"], "psignT")
    onesf_d = nc.inline_tensor(consts["onesf"], "onesf")
    onesb_d = nc.inline_tensor(consts["onesb"], "onesb")

    # collective bounce buffers (internal DRAM)
    ag_in = nc.dram_tensor("ag_in", [QLC, QLEN], F32)
    ag_out = nc.dram_tensor("ag_out", [QL, QLEN], F32, addr_space="Shared")
    rs_in = nc.dram_tensor("rs_in", [QLEN, HID], F32)
    rs_out = nc.dram_tensor("rs_out", [TSH, HID], F32)
    RG = [list(range(NCORES))]

    with tile.TileContext(nc, num_cores=NCORES) as tc:
        with (
            tc.tile_pool(name="const", bufs=1) as constp,
            tc.tile_pool(name="ckv", bufs=1) as ckvp,
            tc.tile_pool(name="qts", bufs=1) as qtsp,
            tc.tile_pool(name="o16", bufs=1) as o16p,
            tc.tile_pool(name="vdec", bufs=3) as vp,
            tc.tile_pool(name="keff", bufs=4) as kp,
            tc.tile_pool(name="abt", bufs=3) as abp,
            tc.tile_pool(name="psV", bufs=1, space="PSUM") as psV,
            tc.tile_pool(name="psP", bufs=2, space="PSUM") as psP,
        ):
            cos2_s = constp.tile([128, QLEN], F32, tag="cos2")
            sin2_s = constp.tile([128, QLEN], F32, tag="sin2")
            psign_s = constp.tile([128, 128], BF16, tag="psign")
            onesf_s = constp.tile([128, 128], F32, tag="onesf")
            onesb_s = constp.tile([128, 1], BF16, tag="onesb")
            nc.sync.dma_start(cos2_s[:], cos2_d[:, :])
            nc.sync.dma_start(sin2_s[:], sin2_d[:, :])
            nc.sync.dma_start(psign_s[:], psignT_d[:, :])
            nc.sync.dma_start(onesf_s[:], onesf_d[:, :])
            nc.sync.dma_start(onesb_s[:], onesb_d[:, :])
            onesf_r = constp.tile([128, 128], F32R, tag="onesf_r")
            nc.vector.tensor_copy(onesf_r[:], onesf_s[:])

            ckv_s = ckvp.tile([128, 5, KVLEN], BF16)  # 4 c-chunks + [kpe;kpe]
            nc.sync.dma_start(
                ckv_s[:], ckv5[:, :].rearrange("(c p) k -> p c k", p=128)
            )

            qts = qtsp.tile([128, NJC, QLEN], BF16)      # scaled q.T
            o16 = o16p.tile([128, HPC, QLEN], BF16)      # normalized per-head o.T

            ab_tiles = {}

            def load_abt(p):
                t = abp.tile([128, NCC, 4 * VD], BF16, tag="abt")
                nc.sync.dma_start(
                    t[:], abT[p].rearrange("(c p) f -> p c f", p=128)
                )
                return t

            def v_decomp(p, ab_s):
                """Decompress V for pair p -> v tile [128k, kc, 2*VD] bf16."""
                v_s = vp.tile([128, NKC, 2 * VD], BF16, tag="v")
                for kc in range(NKC):
                    vps = psV.tile([128, 2 * VD], F32, tag="vps")
                    for ci in range(NCC):
                        nc.tensor.matmul(
                            vps[:],
                            ckv_s[:, ci, kc * 128:(kc + 1) * 128],
                            ab_s[:, ci, 256:512],
                            start=(ci == 0), stop=(ci == NCC - 1),
                        )
                    nc.vector.tensor_copy(v_s[:, kc, :], vps[:])
                return v_s

            def k_eff(hh, ab_s):
                """Decompressed nope-keys for one head: [128d, ks, 512k] bf16."""
                k_s = kp.tile([128, 4, QLEN], BF16, tag="keff")
                for ks in range(4):
                    kq = psP.tile([128, QLEN], F32, tag="pp")
                    for ci in range(NCC):
                        nc.tensor.matmul(
                            kq[:],
                            ab_s[:, ci, 128 * hh:128 * (hh + 1)],
                            ckv_s[:, ci, ks * 512:(ks + 1) * 512],
                            start=(ci == 0), stop=(ci == NCC - 1),
                        )
                    nc.vector.tensor_copy(k_s[:, ks, :], kq[:])
                return k_s

            # ---------------- stage A: q = hsq @ Wqa.T (sharded) -------------
            with (
                tc.tile_pool(name="stgA", bufs=3) as sap,
                tc.tile_pool(name="psA", bufs=1, space="PSUM") as psA,
            ):
                qra = psA.tile([128, QLEN], F32, tag="qra")
                qrb = psA.tile([64, QLEN], F32, tag="qrb")
                for kc in range(NHID):
                    ht = sap.tile([128, QLEN], F32R, tag="ht")
                    nc.sync.dma_start(ht[:], hsqT[kc * 128:(kc + 1) * 128, :])
                    wt = sap.tile([128, QLC], F32R, tag="wt")
                    nc.sync.dma_start(wt[:], wqaT[kc * 128:(kc + 1) * 128, :])
                    nc.tensor.matmul(
                        qra[:], wt[:, 0:128], ht[:],
                        start=(kc == 0), stop=(kc == NHID - 1),
                    )
                    nc.tensor.matmul(
                        qrb[:], wt[:, 128:QLC], ht[:],
                        start=(kc == 0), stop=(kc == NHID - 1),
                    )
                sqa = sap.tile([128, QLEN], F32, tag="sqa")
                sqb = sap.tile([64, QLEN], F32, tag="sqb")
                nc.scalar.copy(sqa[:], qra[:])
                nc.scalar.copy(sqb[:], qrb[:])
                nc.sync.dma_start(ag_in[0:128, :], sqa[:])
                nc.sync.dma_start(ag_in[128:QLC, :], sqb[:])

            nc.gpsimd.collective_compute(
                "AllGather", mybir.AluOpType.bypass,
                replica_groups=RG,
                ins=[ag_in[:, :].opt()], outs=[ag_out[:, :].opt()],
            )

            # q-independent work fills the AllGather wait: V and K
            # decompression for the first pairs
            v_tiles = {}
            keff_tiles = {}
            for p in (0, 1, 2):
                ab_tiles[p] = load_abt(p)
            for p in (0, 1):
                v_tiles[p] = v_decomp(p, ab_tiles[p])
            for h in (0, 1, 2, 3):
                keff_tiles[h] = k_eff(h % 2, ab_tiles[h // 2])

            # -------- stage B: rms scale fold:  qts = qraw * bcast(r) --------
            with (
                tc.tile_pool(name="stgB", bufs=2) as sbp,
                tc.tile_pool(name="stgBs", bufs=1) as sbs,
                tc.tile_pool(name="psB", bufs=1, space="PSUM") as psB,
            ):
                qraw = sbs.tile([128, NJC, QLEN], F32, tag="qraw")
                nc.sync.dma_start(
                    qraw[:], ag_out[:, :].rearrange("(c p) t -> p c t", p=128)
                )
                ssq = psB.tile([1, QLEN], F32, tag="ssq")
                for jc in range(NJC):
                    sq = sbp.tile([128, QLEN], F32R, tag="sq")
                    nc.vector.tensor_mul(sq[:], qraw[:, jc, :], qraw[:, jc, :])
                    nc.tensor.matmul(
                        ssq[:], onesf_r[:, 0:1], sq[:],
                        start=(jc == 0), stop=(jc == NJC - 1),
                    )
                # r_scaled = (1/sqrt(192)) * rsqrt(ssq/1536 + eps)
                #          = 1 / sqrt(ssq*0.125 + 192*eps)
                eps_s = sbs.tile([1, 1], F32, tag="eps")
                nc.gpsimd.memset(eps_s[:], QHD * EPS)
                sqv = sbs.tile([1, QLEN], F32, tag="sqv")
                nc.scalar.activation(
                    sqv[:], ssq[:], AF.Sqrt, scale=QHD / QL, bias=eps_s[:]
                )
                rsc = sbs.tile([1, QLEN], F32R, tag="rsc")
                with nc.allow_low_precision(reason="fp32r"):
                    nc.vector.reciprocal(rsc[:], sqv[:])
                bc = psB.tile([128, QLEN], F32, tag="bc")
                nc.tensor.matmul(
                    bc[:], onesf_r[0:1, :], rsc[:],
                    start=True, stop=True,
                )
                bcs = sbs.tile([128, QLEN], F32, tag="bcs")
                nc.scalar.copy(bcs[:], bc[:])
                for jc in range(NJC):
                    nc.vector.tensor_mul(qts[:, jc, :], qraw[:, jc, :], bcs[:])

            # ---------------- per-pair attention ----------------
            with (
                tc.tile_pool(name="qbt", bufs=2) as qbtp,
                tc.tile_pool(name="hsb", bufs=2) as hsb,     # per-pair sbuf
                tc.tile_pool(name="expp", bufs=4) as expp,
                tc.tile_pool(name="sml", bufs=2) as sml,
                tc.tile_pool(name="psL", bufs=2, space="PSUM") as psL,
                tc.tile_pool(name="psO", bufs=2, space="PSUM") as psO,
                tc.tile_pool(name="psS", bufs=1, space="PSUM") as psS,
            ):
                def load_qbt(p):
                    t = qbtp.tile([128, NJC, 384], BF16, tag="qbt")
                    nc.sync.dma_start(
                        t[:], qbt[p].rearrange("(c p) f -> p c f", p=128)
                    )
                    return t

                def pair_qproj(qbt_s):
                    """-> (qn_sb[2], roped) for the pair."""
                    qn_sb = []
                    for part in range(2):   # nope head a, nope head b
                        qn_ps = psP.tile([128, QLEN], F32, tag="pp")
                        for jc in range(NJC):
                            nc.tensor.matmul(
                                qn_ps[:],
                                qbt_s[:, jc, 128 * part:128 * (part + 1)],
                                qts[:, jc, :],
                                start=(jc == 0), stop=(jc == NJC - 1),
                            )
                        s = hsb.tile([128, QLEN], BF16, tag="qn")
                        nc.scalar.copy(s[:], qn_ps[:])
                        qn_sb.append(s)
                    pe_ps = psP.tile([128, QLEN], F32, tag="pp")
                    for jc in range(NJC):
                        nc.tensor.matmul(
                            pe_ps[:], qbt_s[:, jc, 256:384],
                            qts[:, jc, :],
                            start=(jc == 0), stop=(jc == NJC - 1),
                        )
                    pe_sb = hsb.tile([128, QLEN], BF16, tag="pe")
                    nc.scalar.copy(pe_sb[:], pe_ps[:])
                    rot_ps = psP.tile([128, QLEN], F32, tag="pp")
                    nc.tensor.matmul(
                        rot_ps[:], psign_s[:], pe_sb[:],
                        start=True, stop=True,
                    )
                    tmp1 = hsb.tile([128, QLEN], F32, tag="tmp1")
                    nc.vector.tensor_mul(tmp1[:], pe_sb[:], cos2_s[:])
                    tmp2 = hsb.tile([128, QLEN], F32, tag="tmp2")
                    nc.vector.tensor_mul(tmp2[:], rot_ps[:], sin2_s[:])
                    roped = hsb.tile([128, QLEN], BF16, tag="roped")
                    nc.vector.tensor_add(roped[:], tmp1[:], tmp2[:])
                    return qn_sb, roped

                def pair_attn(p, qn_sb, roped, v_s, keffs):
                    """Merged kc loop over both heads of the pair.

                    Both heads' softmax row-sums accumulate in ONE psum bank
                    at disjoint partitions (0 and 64) so every downstream
                    consumer stays partition-aligned."""
                    oT = [psO.tile([128, QLEN], F32, tag="ot", name=f"oT{p}_{i}")
                          for i in range(2)]
                    ssum2 = psS.tile([65, QLEN], F32, tag="ssum")
                    ssum = [ssum2[0:1, :], ssum2[64:65, :]]
                    for kc in range(NKC):
                        lg = [psL.tile([128, QLEN], F32, tag="lg",
                                       name=f"lg{p}_{kc}_{i}") for i in range(2)]
                        for hh in range(2):
                            nc.tensor.matmul(
                                lg[hh][:],
                                keffs[hh][:, kc // 4,
                                          (kc % 4) * 128:(kc % 4 + 1) * 128],
                                qn_sb[hh][:],
                                start=True, stop=False,
                            )
                        # the two K=64 rope matmuls target disjoint PE row
                        # groups (0:64 / 64:128) and run concurrently
                        for hh in range(2):
                            nc.tensor.matmul(
                                lg[hh][:],
                                ckv_s[:, 4, kc * 128:(kc + 1) * 128][
                                    64 * hh:64 * (hh + 1), :],
                                roped[64 * hh:64 * (hh + 1), :],
                                start=False, stop=True,
                            )
                        for hh in range(2):
                            ex = expp.tile([128, QLEN], BF16, tag="ex")
                            nc.scalar.activation(ex[:], lg[hh][:], AF.Exp)
                            nc.tensor.matmul(
                                oT[hh][:],
                                v_s[:, kc, VD * hh:VD * (hh + 1)],
                                ex[:],
                                start=(kc == 0), stop=(kc == NKC - 1),
                            )
                            nc.tensor.matmul(
                                ssum[hh], onesb_s[:], ex[:],
                                start=(kc == 0), stop=(kc == NKC - 1),
                                skip_group_check=True,
                            )
                    invf2 = sml.tile([65, QLEN], F32, tag="invf")
                    inv2 = sml.tile([65, QLEN], F32R, tag="inv")
                    for hh in range(2):
                        row = slice(64 * hh, 64 * hh + 1)
                        with nc.allow_low_precision(reason="fp32r"):
                            nc.vector.reciprocal(inv2[row, :], ssum[hh])
                        bcp = psP.tile([128, QLEN], F32, tag="pp")
                        nc.tensor.matmul(
                            bcp[:], onesf_r[row, :], inv2[row, :],
                            start=True, stop=True,
                        )
                        binv = sml.tile([128, QLEN], F32, tag="binv")
                        nc.scalar.copy(binv[:], bcp[:])
                        nc.vector.tensor_mul(
                            o16[:, 2 * p + hh, :], oT[hh][:], binv[:]
                        )

                qbt_tiles = {0: load_qbt(0)}
                for p in range(PAIRS):
                    qbt_s = qbt_tiles.pop(p)
                    if p + 1 < PAIRS:
                        qbt_tiles[p + 1] = load_qbt(p + 1)
                    qn_sb, roped = pair_qproj(qbt_s)
                    v_s = v_tiles.pop(p)
                    keffs = [keff_tiles.pop(2 * p), keff_tiles.pop(2 * p + 1)]
                    pair_attn(p, qn_sb, roped, v_s, keffs)
                    # prefetch decompression for later pairs
                    ab_tiles.pop(p, None)
                    if p + 3 < PAIRS:
                        ab_tiles[p + 3] = load_abt(p + 3)
                    if p + 2 < PAIRS:
                        v_tiles[p + 2] = v_decomp(p + 2, ab_tiles[p + 2])
                        keff_tiles[2 * p + 4] = k_eff(0, ab_tiles[p + 2])
                        keff_tiles[2 * p + 5] = k_eff(1, ab_tiles[p + 2])

            # ---------------- output projection + ReduceScatter --------------
            with (
                tc.tile_pool(name="wo", bufs=2) as wop,
                tc.tile_pool(name="osb", bufs=3) as osb,
                tc.tile_pool(name="psW", bufs=2, space="PSUM") as psW,
            ):
                for ds in range(NDS):
                    w16 = wop.tile([128, HPC, 512], BF16, tag="w16")
                    nc.sync.dma_start(
                        w16[:],
                        woT[:, :].rearrange("(g p) d -> p g d", p=128)[
                            :, :, ds * 512:(ds + 1) * 512],
                    )
                    for tc4 in range(4):
                        acc = psW.tile([128, 512], F32, tag="acc")
                        for g in range(HPC):
                            nc.tensor.matmul(
                                acc[:],
                                o16[:, g, tc4 * 128:(tc4 + 1) * 128],
                                w16[:, g, :],
                                start=(g == 0), stop=(g == HPC - 1),
                            )
                        ot = osb.tile([128, 512], F32, tag="ot")
                        nc.vector.tensor_copy(ot[:], acc[:])
                        nc.sync.dma_start(
                            rs_in[tc4 * 128:(tc4 + 1) * 128,
                                  ds * 512:(ds + 1) * 512],
                            ot[:],
                        )

            nc.gpsimd.collective_compute(
                "ReduceScatter", mybir.AluOpType.add,
                replica_groups=RG,
                ins=[rs_in[:, :].opt()], outs=[rs_out[:, :].opt()],
            )
            nc.gpsimd.dma_start(out_sh[:, :], rs_out[:, :])

    nc.compile()
    return nc


_CACHE = {}


def _get_program(consts):
    key = (consts["cos2"].tobytes(), consts["sin2"].tobytes())
    if key not in _CACHE:
        _CACHE[key] = _build_program(consts)
    return _CACHE[key]


def _run(inputs, **kwargs):
    in_maps, consts = _host_prepare(inputs)
    nc = _get_program(consts)
    res = run_bass_kernel_spmd(nc, in_maps, core_ids=list(range(NCORES)), **kwargs)
    shards = [res.results[c]["out_shard"] for c in range(NCORES)]
    out = np.concatenate(shards, axis=0)[None].astype(np.float32)
    return out, res


def kernel(**inputs) -> np.ndarray:
    return _run(inputs)[0]


# revision 17
# speedup vs baseline: 1.2795x; 1.0403x over previous
"""DeepSeek-V2 MLA attention (weight-absorbed) on 8 Trainium2 NeuronCores.

Sharding: tensor-parallel over the 128 heads (16 heads/core).  The q
LoRA projection (hidden @ Wqa.T) is sharded over the Q_LORA output dim
and AllGathered; the per-head attention runs fully local; the output
projection partials are summed with a ReduceScatter over the token axis
and the 8 shards are concatenated on the host.

Math restructuring vs the reference (exactly associativity-equivalent):
  - q_nope = (q @ qb_nope.T) @ q_absorb          (factor through the 128-dim)
  - o      = softmax(l) @ (ckv @ out_absorb.T)   (decompress V, 128-dim)
  - out    = concat_h(o_h) @ Wo.T                (plain o_proj)
  - rmsnorm's per-token scale and 1/sqrt(192) are folded into q;
    qa_ln_w is folded into Wqb; the RoPE interleave permutation is
    folded into the rope rows of Wqb; softmax skips the max-subtraction
    (logits are O(3) for this problem) and normalizes o after PV.
"""

import math
import numpy as np
import ml_dtypes

import concourse.bass as bass
import concourse.bacc as bacc
import concourse.mybir as mybir
import concourse.tile as tile
from concourse.bass_utils import run_bass_kernel_spmd

F32 = mybir.dt.float32
F32R = mybir.dt.float32r
BF16 = mybir.dt.bfloat16
AF = mybir.ActivationFunctionType

H, QL, KL, ROPE, NOPE, VD, HID = 128, 1536, 512, 64, 128, 128, 5120
QHD = NOPE + ROPE  # 192
QLEN, KVLEN = 512, 2048
NCORES = 8
HPC = H // NCORES          # 16 heads per core
PAIRS = HPC // 2           # 8 pairs per core
QLC = QL // NCORES         # 192 q-lora rows per core
TSH = QLEN // NCORES       # 64 token rows per output shard
NKC = KVLEN // 128         # 16 kv chunks
NJC = QL // 128            # 12 q-lora chunks
NCC = KL // 128            # 4 compressed-kv chunks
NHID = HID // 128          # 40 hidden chunks
NDS = HID // 512           # 10 output dim slices
EPS = 1e-6





def _host_prepare(inputs):
    """Full inputs -> (list of per-core input dicts, const arrays)."""
    hsq = np.asarray(inputs["hidden_states_q"], np.float32)[0]      # [512, 5120]
    pos = np.asarray(inputs["q_position_ids"])[0]                   # [512]
    ckv_full = np.asarray(inputs["compressed_kv"], np.float32)[0]   # [2048, 576]
    Wqa = np.asarray(inputs["Wqa"], np.float32)                     # [1536, 5120]
    w_ln = np.asarray(inputs["qa_ln_w"], np.float32)                # [1536]
    Wqb = np.asarray(inputs["Wqb"], np.float32)                     # [24576, 1536]
    Wkvb = np.asarray(inputs["Wkvb"], np.float32)                   # [32768, 512]
    Wo = np.asarray(inputs["Wo"], np.float32)                       # [5120, 16384]

    hsqT = np.ascontiguousarray(hsq.T)                              # [5120, 512]
    ckvT = np.ascontiguousarray(ckv_full.T)                         # [576, 2048]
    kpeT = ckvT[KL:]                                                # [64, 2048]
    # c chunks + k_pe duplicated twice (so both heads of a pair can use
    # partition-aligned lhsT slices at base 0 / 64)
    ckv5 = np.concatenate([ckvT[:KL], kpeT, kpeT], axis=0).astype(
        ml_dtypes.bfloat16)                                         # [640, 2048]

    Wqb_w = Wqb * w_ln[None, :]
    qb3 = Wqb_w.reshape(H, QHD, QL)
    kvb = Wkvb.reshape(H, NOPE + VD, KL)
    perm = np.concatenate([np.arange(0, ROPE, 2), np.arange(1, ROPE, 2)])

    # rope tables in half-split layout, [d, t]; doubled over the pair axis
    inv_freq = 1.0 / (10000.0 ** (np.arange(0, ROPE, 2, dtype=np.float64) / ROPE))
    fr = np.outer(pos.astype(np.float64), inv_freq)                 # [512, 32]
    emb = np.concatenate([fr, fr], axis=-1)                         # [512, 64]
    cosT = np.cos(emb).T.astype(np.float32)                         # [64, 512]
    sinT = np.sin(emb).T.astype(np.float32)
    cos2 = np.ascontiguousarray(np.concatenate([cosT, cosT], axis=0))  # [128, 512]
    sin2 = np.ascontiguousarray(np.concatenate([sinT, sinT], axis=0))

    # rot = blockdiag(P, P) @ q'   with  rot_h = [-q'[32:], q'[:32]]
    P64 = np.zeros((ROPE, ROPE), np.float32)
    P64[np.arange(32), np.arange(32) + 32] = -1.0
    P64[np.arange(32, 64), np.arange(32)] = 1.0
    psign = np.zeros((128, 128), np.float32)
    psign[:64, :64] = P64
    psign[64:, 64:] = P64
    psignT = np.ascontiguousarray(psign.T)

    consts = {
        "cos2": cos2,
        "sin2": sin2,
        "psignT": psignT.astype(ml_dtypes.bfloat16),
        "onesf": np.ones((128, 128), np.float32),
        "onesb": np.ones((128, 1), ml_dtypes.bfloat16),
    }

    in_maps = []
    for c in range(NCORES):
        h0 = c * HPC
        wqaT = np.ascontiguousarray(Wqa[c * QLC:(c + 1) * QLC].T)   # [5120, 192]
        qbt = np.empty((PAIRS, QL, 384), ml_dtypes.bfloat16)
        abT = np.empty((PAIRS, KL, 4 * VD), ml_dtypes.bfloat16)
        for p in range(PAIRS):
            ha, hb = h0 + 2 * p, h0 + 2 * p + 1
            qbt[p, :, 0:128] = qb3[ha, :NOPE].T
            qbt[p, :, 128:256] = qb3[hb, :NOPE].T
            qbt[p, :, 256:320] = qb3[ha, NOPE:][perm].T
            qbt[p, :, 320:384] = qb3[hb, NOPE:][perm].T
            abT[p, :, 0:128] = kvb[ha, :NOPE].T      # q_absorb.T head a
            abT[p, :, 128:256] = kvb[hb, :NOPE].T    # q_absorb.T head b
            abT[p, :, 256:384] = kvb[ha, NOPE:].T    # out_absorb.T head a
            abT[p, :, 384:512] = kvb[hb, NOPE:].T    # out_absorb.T head b
        woT = np.ascontiguousarray(
            Wo[:, h0 * VD:(h0 + HPC) * VD].T
        ).astype(ml_dtypes.bfloat16)                                # [2048, 5120]
        in_maps.append({
            "hsqT": hsqT,
            "wqaT": wqaT,
            "qbt": qbt,
            "abT": abT,
            "ckv5": ckv5,
            "woT": woT,
        })
    return in_maps, consts


def _build_program(consts):
    nc = bacc.Bacc("TRN2", num_devices=NCORES)

    hsqT = nc.dram_tensor("hsqT", [HID, QLEN], F32R, kind="ExternalInput")
    wqaT = nc.dram_tensor("wqaT", [HID, QLC], F32R, kind="ExternalInput")
    qbt = nc.dram_tensor("qbt", [PAIRS, QL, 384], BF16, kind="ExternalInput")
    abT = nc.dram_tensor("abT", [PAIRS, KL, 4 * VD], BF16, kind="ExternalInput")
    ckv5 = nc.dram_tensor("ckv5", [640, KVLEN], BF16, kind="ExternalInput")
    woT = nc.dram_tensor("woT", [HPC * VD, HID], BF16, kind="ExternalInput")
    out_sh = nc.dram_tensor("out_shard", [TSH, HID], F32, kind="ExternalOutput")

    cos2_d = nc.inline_tensor(consts["cos2"], "cos2")
    sin2_d = nc.inline_tensor(consts["sin2"], "sin2")
    psignT_d = nc.inline_tensor(consts["psignT"], "psignT")
    onesf_d = nc.inline_tensor(consts["onesf"], "onesf")
    onesb_d = nc.inline_tensor(consts["onesb"], "onesb")

    # collective bounce buffers (internal DRAM)
    ag_in = nc.dram_tensor("ag_in", [QLC, QLEN], F32)
    ar_in = nc.dram_tensor("ar_in", [1, QLEN], F32)
    ar_out = nc.dram_tensor("ar_out", [1, QLEN], F32)
    ag_out = nc.dram_tensor("ag_out", [QL, QLEN], F32, addr_space="Shared")
    rs_in = nc.dram_tensor("rs_in", [QLEN, HID], F32)
    rs_out = nc.dram_tensor("rs_out", [TSH, HID], F32)
    RG = [list(range(NCORES))]

    with tile.TileContext(nc, num_cores=NCORES) as tc:
        with (
            tc.tile_pool(name="const", bufs=1) as constp,
            tc.tile_pool(name="ckv", bufs=1) as ckvp,
            tc.tile_pool(name="qts", bufs=1) as qtsp,
            tc.tile_pool(name="o16", bufs=1) as o16p,
            tc.tile_pool(name="vdec", bufs=3) as vp,
            tc.tile_pool(name="keff", bufs=6) as kp,
            tc.tile_pool(name="abt", bufs=3) as abp,
            tc.tile_pool(name="psV", bufs=1, space="PSUM") as psV,
            tc.tile_pool(name="psP", bufs=2, space="PSUM") as psP,
        ):
            cos2_s = constp.tile([128, QLEN], F32, tag="cos2")
            sin2_s = constp.tile([128, QLEN], F32, tag="sin2")
            psign_s = constp.tile([128, 128], BF16, tag="psign")
            onesf_s = constp.tile([128, 128], F32, tag="onesf")
            onesb_s = constp.tile([128, 1], BF16, tag="onesb")
            nc.sync.dma_start(cos2_s[:], cos2_d[:, :])
            nc.sync.dma_start(sin2_s[:], sin2_d[:, :])
            nc.sync.dma_start(psign_s[:], psignT_d[:, :])
            nc.sync.dma_start(onesf_s[:], onesf_d[:, :])
            nc.sync.dma_start(onesb_s[:], onesb_d[:, :])
            onesf_r = constp.tile([128, 128], F32R, tag="onesf_r")
            nc.vector.tensor_copy(onesf_r[:], onesf_s[:])

            ckv_s = ckvp.tile([128, 5, KVLEN], BF16)  # 4 c-chunks + [kpe;kpe]
            nc.sync.dma_start(
                ckv_s[:], ckv5[:, :].rearrange("(c p) k -> p c k", p=128)
            )

            qts = qtsp.tile([128, NJC, QLEN], BF16)      # scaled q.T
            o16 = o16p.tile([128, HPC, QLEN], BF16)      # normalized per-head o.T

            ab_tiles = {}

            def load_abt(p):
                t = abp.tile([128, NCC, 4 * VD], BF16, tag="abt")
                nc.sync.dma_start(
                    t[:], abT[p].rearrange("(c p) f -> p c f", p=128)
                )
                return t

            def v_decomp(p, ab_s):
                """Decompress V for pair p -> v tile [128k, kc, 2*VD] bf16."""
                v_s = vp.tile([128, NKC, 2 * VD], BF16, tag="v")
                for kc in range(NKC):
                    vps = psV.tile([128, 2 * VD], F32, tag="vps")
                    for ci in range(NCC):
                        nc.tensor.matmul(
                            vps[:],
                            ckv_s[:, ci, kc * 128:(kc + 1) * 128],
                            ab_s[:, ci, 256:512],
                            start=(ci == 0), stop=(ci == NCC - 1),
                        )
                    nc.vector.tensor_copy(v_s[:, kc, :], vps[:])
                return v_s

            def k_eff(hh, ab_s):
                """Decompressed nope-keys for one head: [128d, ks, 512k] bf16."""
                k_s = kp.tile([128, 4, QLEN], BF16, tag="keff")
                for ks in range(4):
                    kq = psP.tile([128, QLEN], F32, tag="pp")
                    for ci in range(NCC):
                        nc.tensor.matmul(
                            kq[:],
                            ab_s[:, ci, 128 * hh:128 * (hh + 1)],
                            ckv_s[:, ci, ks * 512:(ks + 1) * 512],
                            start=(ci == 0), stop=(ci == NCC - 1),
                        )
                    nc.vector.tensor_copy(k_s[:, ks, :], kq[:])
                return k_s

            # ---------------- stage A: q = hsq @ Wqa.T (sharded) -------------
            with (
                tc.tile_pool(name="stgA", bufs=3) as sap,
                tc.tile_pool(name="psA", bufs=1, space="PSUM") as psA,
            ):
                qra = psA.tile([128, QLEN], F32, tag="qra")
                qrb = psA.tile([64, QLEN], F32, tag="qrb")
                for kc in range(NHID):
                    ht = sap.tile([128, QLEN], F32R, tag="ht")
                    nc.sync.dma_start(ht[:], hsqT[kc * 128:(kc + 1) * 128, :])
                    wt = sap.tile([128, QLC], F32R, tag="wt")
                    nc.sync.dma_start(wt[:], wqaT[kc * 128:(kc + 1) * 128, :])
                    nc.tensor.matmul(
                        qra[:], wt[:, 0:128], ht[:],
                        start=(kc == 0), stop=(kc == NHID - 1),
                    )
                    nc.tensor.matmul(
                        qrb[:], wt[:, 128:QLC], ht[:],
                        start=(kc == 0), stop=(kc == NHID - 1),
                    )
                sqa = sap.tile([128, QLEN], F32, tag="sqa")
                sqb = sap.tile([64, QLEN], F32, tag="sqb")
                nc.scalar.copy(sqa[:], qra[:])
                nc.scalar.copy(sqb[:], qrb[:])
                nc.sync.dma_start(ag_in[0:128, :], sqa[:])
                nc.sync.dma_start(ag_in[128:QLC, :], sqb[:])
                # local partial sum-of-squares over this core's 192 q rows
                sq2a = sap.tile([128, QLEN], F32R, tag="sq2a")
                sq2b = sap.tile([64, QLEN], F32R, tag="sq2b")
                nc.vector.tensor_mul(sq2a[:], sqa[:], sqa[:])
                nc.vector.tensor_mul(sq2b[:], sqb[:], sqb[:])
                ssqp = psA.tile([1, QLEN], F32, tag="ssqp")
                nc.tensor.matmul(
                    ssqp[:], onesf_r[:, 0:1], sq2a[:], start=True, stop=False
                )
                nc.tensor.matmul(
                    ssqp[:], onesf_r[0:64, 0:1], sq2b[:], start=False, stop=True
                )
                ssq_s = sap.tile([1, QLEN], F32, tag="ssq_s")
                nc.scalar.copy(ssq_s[:], ssqp[:])
                nc.sync.dma_start(ar_in[:, :], ssq_s[:])

            nc.gpsimd.collective_compute(
                "AllReduce", mybir.AluOpType.add,
                replica_groups=RG,
                ins=[ar_in[:, :].opt()], outs=[ar_out[:, :].opt()],
            )
            nc.gpsimd.collective_compute(
                "AllGather", mybir.AluOpType.bypass,
                replica_groups=RG,
                ins=[ag_in[:, :].opt()], outs=[ag_out[:, :].opt()],
            )

            # q-independent work fills the collective wait: V and K
            # decompression for the first pairs
            v_tiles = {}
            keff_tiles = {}
            for p in (0, 1, 2):
                ab_tiles[p] = load_abt(p)
            for p in (0, 1):
                v_tiles[p] = v_decomp(p, ab_tiles[p])
            for h in range(6):
                keff_tiles[h] = k_eff(h % 2, ab_tiles[h // 2])

            # -------- stage B: rms scale fold:  qts = qraw * bcast(r) --------
            with (
                tc.tile_pool(name="stgBs", bufs=1) as sbs,
            ):
                # r_scaled = (1/sqrt(192)) * rsqrt(ssq/1536 + eps)
                #          = 1 / sqrt(ssq*0.125 + 192*eps)
                ssqf = sbs.tile([1, QLEN], F32, tag="ssqf")
                nc.sync.dma_start(ssqf[:], ar_out[:, :])
                eps_s = sbs.tile([1, 1], F32, tag="eps")
                nc.gpsimd.memset(eps_s[:], QHD * EPS)
                sqv = sbs.tile([1, QLEN], F32, tag="sqv")
                nc.scalar.activation(
                    sqv[:], ssqf[:], AF.Sqrt, scale=QHD / QL, bias=eps_s[:]
                )
                rsc = sbs.tile([1, QLEN], F32, tag="rsc")
                nc.vector.reciprocal(rsc[:], sqv[:])
                bcs = sbs.tile([128, QLEN], F32, tag="bcs")
                nc.gpsimd.partition_broadcast(bcs[:], rsc[:])
                qraw = sbs.tile([128, NJC, QLEN], F32, tag="qraw")
                nc.sync.dma_start(
                    qraw[:], ag_out[:, :].rearrange("(c p) t -> p c t", p=128)
                )
                for jc in range(NJC):
                    nc.vector.tensor_mul(qts[:, jc, :], qraw[:, jc, :], bcs[:])

            # ---------------- per-pair attention ----------------
            with (
                tc.tile_pool(name="qbt", bufs=2) as qbtp,
                tc.tile_pool(name="hsb", bufs=2) as hsb,     # per-pair sbuf
                tc.tile_pool(name="expp", bufs=4) as expp,
                tc.tile_pool(name="sml", bufs=2) as sml,
                tc.tile_pool(name="psL", bufs=2, space="PSUM") as psL,
                tc.tile_pool(name="psO", bufs=2, space="PSUM") as psO,
                tc.tile_pool(name="psS", bufs=1, space="PSUM") as psS,
            ):
                def load_qbt(p):
                    t = qbtp.tile([128, NJC, 384], BF16, tag="qbt")
                    nc.sync.dma_start(
                        t[:], qbt[p].rearrange("(c p) f -> p c f", p=128)
                    )
                    return t

                def pair_qproj(qbt_s):
                    """-> (qn_sb[2], roped) for the pair."""
                    qn_sb = []
                    for part in range(2):   # nope head a, nope head b
                        qn_ps = psP.tile([128, QLEN], F32, tag="pp")
                        for jc in range(NJC):
                            nc.tensor.matmul(
                                qn_ps[:],
                                qbt_s[:, jc, 128 * part:128 * (part + 1)],
                                qts[:, jc, :],
                                start=(jc == 0), stop=(jc == NJC - 1),
                            )
                        s = hsb.tile([128, QLEN], BF16, tag="qn")
                        nc.scalar.copy(s[:], qn_ps[:])
                        qn_sb.append(s)
                    pe_ps = psP.tile([128, QLEN], F32, tag="pp")
                    for jc in range(NJC):
                        nc.tensor.matmul(
                            pe_ps[:], qbt_s[:, jc, 256:384],
                            qts[:, jc, :],
                            start=(jc == 0), stop=(jc == NJC - 1),
                        )
                    pe_sb = hsb.tile([128, QLEN], BF16, tag="pe")
                    nc.scalar.copy(pe_sb[:], pe_ps[:])
                    rot_ps = psP.tile([128, QLEN], F32, tag="pp")
                    nc.tensor.matmul(
                        rot_ps[:], psign_s[:], pe_sb[:],
                        start=True, stop=True,
                    )
                    tmp1 = hsb.tile([128, QLEN], F32, tag="tmp1")
                    nc.vector.tensor_mul(tmp1[:], pe_sb[:], cos2_s[:])
                    tmp2 = hsb.tile([128, QLEN], F32, tag="tmp2")
                    nc.vector.tensor_mul(tmp2[:], rot_ps[:], sin2_s[:])
                    roped = hsb.tile([128, QLEN], BF16, tag="roped")
                    nc.vector.tensor_add(roped[:], tmp1[:], tmp2[:])
                    return qn_sb, roped

                def pair_attn(p, qn_sb, roped, v_s, keffs):
                    """Merged kc loop over both heads of the pair.

                    Both heads' softmax row-sums accumulate in ONE psum bank
                    at disjoint partitions (0 and 64) so every downstream
                    consumer stays partition-aligned."""
                    oT = [psO.tile([128, QLEN], F32, tag="ot", name=f"oT{p}_{i}")
                          for i in range(2)]
                    ssum2 = psS.tile([65, QLEN], F32, tag="ssum")
                    ssum = [ssum2[0:1, :], ssum2[64:65, :]]
                    for kc in range(NKC):
                        lg = [psL.tile([128, QLEN], F32, tag="lg",
                                       name=f"lg{p}_{kc}_{i}") for i in range(2)]
                        for hh in range(2):
                            nc.tensor.matmul(
                                lg[hh][:],
                                keffs[hh][:, kc // 4,
                                          (kc % 4) * 128:(kc % 4 + 1) * 128],
                                qn_sb[hh][:],
                                start=True, stop=False,
                            )
                        # the two K=64 rope matmuls target disjoint PE row
                        # groups (0:64 / 64:128) and run concurrently
                        for hh in range(2):
                            nc.tensor.matmul(
                                lg[hh][:],
                                ckv_s[:, 4, kc * 128:(kc + 1) * 128][
                                    64 * hh:64 * (hh + 1), :],
                                roped[64 * hh:64 * (hh + 1), :],
                                start=False, stop=True,
                            )
                        for hh in range(2):
                            ex = expp.tile([128, QLEN], BF16, tag="ex")
                            nc.scalar.activation(ex[:], lg[hh][:], AF.Exp)
                            nc.tensor.matmul(
                                oT[hh][:],
                                v_s[:, kc, VD * hh:VD * (hh + 1)],
                                ex[:],
                                start=(kc == 0), stop=(kc == NKC - 1),
                            )
                            nc.tensor.matmul(
                                ssum[hh], onesb_s[:], ex[:],
                                start=(kc == 0), stop=(kc == NKC - 1),
                                skip_group_check=True,
                            )
                    inv2 = sml.tile([65, QLEN], F32, tag="inv")
                    for hh in range(2):
                        row = slice(64 * hh, 64 * hh + 1)
                        nc.vector.reciprocal(inv2[row, :], ssum[hh])
                        if hh == 0:
                            src_row = inv2[0:1, :]
                        else:
                            # partition_broadcast ucode always reads the
                            # physical partition 0 — move the row there first
                            inv_b0 = sml.tile([1, QLEN], F32, tag="inv_b0")
                            nc.sync.dma_start(inv_b0[:], inv2[64:65, :])
                            src_row = inv_b0[:]
                        binv = sml.tile([128, QLEN], F32, tag="binv")
                        nc.gpsimd.partition_broadcast(binv[:], src_row)
                        nc.vector.tensor_mul(
                            o16[:, 2 * p + hh, :], oT[hh][:], binv[:]
                        )

                qbt_tiles = {0: load_qbt(0)}
                for p in range(PAIRS):
                    qbt_s = qbt_tiles.pop(p)
                    if p + 1 < PAIRS:
                        qbt_tiles[p + 1] = load_qbt(p + 1)
                    qn_sb, roped = pair_qproj(qbt_s)
                    v_s = v_tiles.pop(p)
                    keffs = [keff_tiles.pop(2 * p), keff_tiles.pop(2 * p + 1)]
                    pair_attn(p, qn_sb, roped, v_s, keffs)
                    # prefetch decompression for later pairs
                    ab_tiles.pop(p, None)
                    if p + 3 < PAIRS:
                        ab_tiles[p + 3] = load_abt(p + 3)
                    if p + 2 < PAIRS:
                        v_tiles[p + 2] = v_decomp(p + 2, ab_tiles[p + 2])
                    if p + 3 < PAIRS:
                        keff_tiles[2 * p + 6] = k_eff(0, ab_tiles[p + 3])
                        keff_tiles[2 * p + 7] = k_eff(1, ab_tiles[p + 3])

            # ---------------- output projection + ReduceScatter --------------
            with (
                tc.tile_pool(name="wo", bufs=2) as wop,
                tc.tile_pool(name="osb", bufs=3) as osb,
                tc.tile_pool(name="psW", bufs=2, space="PSUM") as psW,
            ):
                for ds in range(NDS):
                    w16 = wop.tile([128, HPC, 512], BF16, tag="w16")
                    nc.sync.dma_start(
                        w16[:],
                        woT[:, :].rearrange("(g p) d -> p g d", p=128)[
                            :, :, ds * 512:(ds + 1) * 512],
                    )
                    for tc4 in range(4):
                        acc = psW.tile([128, 512], F32, tag="acc")
                        for g in range(HPC):
                            nc.tensor.matmul(
                                acc[:],
                                o16[:, g, tc4 * 128:(tc4 + 1) * 128],
                                w16[:, g, :],
                                start=(g == 0), stop=(g == HPC - 1),
                            )
                        ot = osb.tile([128, 512], F32, tag="ot")
                        nc.vector.tensor_copy(ot[:], acc[:])
                        nc.sync.dma_start(
                            rs_in[tc4 * 128:(tc4 + 1) * 128,
                                  ds * 512:(ds + 1) * 512],
                            ot[:],
                        )

            nc.gpsimd.collective_compute(
                "ReduceScatter", mybir.AluOpType.add,
                replica_groups=RG,
                ins=[rs_in[:, :].opt()], outs=[rs_out[:, :].opt()],
            )
            nc.gpsimd.dma_start(out_sh[:, :], rs_out[:, :])

    nc.compile()
    return nc


_CACHE = {}


def _get_program(consts):
    key = (consts["cos2"].tobytes(), consts["sin2"].tobytes())
    if key not in _CACHE:
        _CACHE[key] = _build_program(consts)
    return _CACHE[key]


def _run(inputs, **kwargs):
    in_maps, consts = _host_prepare(inputs)
    nc = _get_program(consts)
    res = run_bass_kernel_spmd(nc, in_maps, core_ids=list(range(NCORES)), **kwargs)
    shards = [res.results[c]["out_shard"] for c in range(NCORES)]
    out = np.concatenate(shards, axis=0)[None].astype(np.float32)
    return out, res


def kernel(**inputs) -> np.ndarray:
    return _run(inputs)[0]


# revision 19
# speedup vs baseline: 1.3670x; 1.0683x over previous
"""DeepSeek-V2 MLA attention (weight-absorbed) on 8 Trainium2 NeuronCores.

Sharding: tensor-parallel over the 128 heads (16 heads/core).  The q
LoRA projection (hidden @ Wqa.T) is sharded over the Q_LORA output dim
and AllGathered; the per-head attention runs fully local; the output
projection partials are summed with a ReduceScatter over the token axis
and the 8 shards are concatenated on the host.

Math restructuring vs the reference (exactly associativity-equivalent):
  - q_nope = (q @ qb_nope.T) @ q_absorb          (factor through the 128-dim)
  - o      = softmax(l) @ (ckv @ out_absorb.T)   (decompress V, 128-dim)
  - out    = concat_h(o_h) @ Wo.T                (plain o_proj)
  - rmsnorm's per-token scale and 1/sqrt(192) are folded into q;
    qa_ln_w is folded into Wqb; the RoPE interleave permutation is
    folded into the rope rows of Wqb; softmax skips the max-subtraction
    (logits are O(3) for this problem) and normalizes o after PV.
"""

import math
import numpy as np
import ml_dtypes

import concourse.bass as bass
import concourse.bacc as bacc
import concourse.mybir as mybir
import concourse.tile as tile
from concourse.bass_utils import run_bass_kernel_spmd

F32 = mybir.dt.float32
F32R = mybir.dt.float32r
BF16 = mybir.dt.bfloat16
AF = mybir.ActivationFunctionType

H, QL, KL, ROPE, NOPE, VD, HID = 128, 1536, 512, 64, 128, 128, 5120
QHD = NOPE + ROPE  # 192
QLEN, KVLEN = 512, 2048
NCORES = 8
HPC = H // NCORES          # 16 heads per core
PAIRS = HPC // 2           # 8 pairs per core
QLC = QL // NCORES         # 192 q-lora rows per core
TSH = QLEN // NCORES       # 64 token rows per output shard
NKC = KVLEN // 128         # 16 kv chunks
NJC = QL // 128            # 12 q-lora chunks
NCC = KL // 128            # 4 compressed-kv chunks
NHID = HID // 128          # 40 hidden chunks
NDS = HID // 512           # 10 output dim slices
EPS = 1e-6





def _host_prepare(inputs):
    """Full inputs -> (list of per-core input dicts, const arrays)."""
    hsq = np.asarray(inputs["hidden_states_q"], np.float32)[0]      # [512, 5120]
    pos = np.asarray(inputs["q_position_ids"])[0]                   # [512]
    ckv_full = np.asarray(inputs["compressed_kv"], np.float32)[0]   # [2048, 576]
    Wqa = np.asarray(inputs["Wqa"], np.float32)                     # [1536, 5120]
    w_ln = np.asarray(inputs["qa_ln_w"], np.float32)                # [1536]
    Wqb = np.asarray(inputs["Wqb"], np.float32)                     # [24576, 1536]
    Wkvb = np.asarray(inputs["Wkvb"], np.float32)                   # [32768, 512]
    Wo = np.asarray(inputs["Wo"], np.float32)                       # [5120, 16384]

    hsqT = np.ascontiguousarray(hsq.T)                              # [5120, 512]
    ckvT = np.ascontiguousarray(ckv_full.T)                         # [576, 2048]
    kpeT = ckvT[KL:]                                                # [64, 2048]
    # c chunks + k_pe duplicated twice (so both heads of a pair can use
    # partition-aligned lhsT slices at base 0 / 64)
    ckv5 = np.concatenate([ckvT[:KL], kpeT, kpeT], axis=0).astype(
        ml_dtypes.bfloat16)                                         # [640, 2048]

    Wqb_w = Wqb * w_ln[None, :]
    qb3 = Wqb_w.reshape(H, QHD, QL)
    kvb = Wkvb.reshape(H, NOPE + VD, KL)
    perm = np.concatenate([np.arange(0, ROPE, 2), np.arange(1, ROPE, 2)])

    # rope tables in half-split layout, [d, t]; doubled over the pair axis
    inv_freq = 1.0 / (10000.0 ** (np.arange(0, ROPE, 2, dtype=np.float64) / ROPE))
    fr = np.outer(pos.astype(np.float64), inv_freq)                 # [512, 32]
    emb = np.concatenate([fr, fr], axis=-1)                         # [512, 64]
    cosT = np.cos(emb).T.astype(np.float32)                         # [64, 512]
    sinT = np.sin(emb).T.astype(np.float32)
    cos2 = np.ascontiguousarray(np.concatenate([cosT, cosT], axis=0))  # [128, 512]
    sin2 = np.ascontiguousarray(np.concatenate([sinT, sinT], axis=0))

    # rot = blockdiag(P, P) @ q'   with  rot_h = [-q'[32:], q'[:32]]
    P64 = np.zeros((ROPE, ROPE), np.float32)
    P64[np.arange(32), np.arange(32) + 32] = -1.0
    P64[np.arange(32, 64), np.arange(32)] = 1.0
    psign = np.zeros((128, 128), np.float32)
    psign[:64, :64] = P64
    psign[64:, 64:] = P64
    psignT = np.ascontiguousarray(psign.T)

    consts = {
        "cos2": cos2,
        "sin2": sin2,
        "psignT": psignT.astype(ml_dtypes.bfloat16),
        "onesf": np.ones((128, 128), np.float32),
        "onesb": np.ones((128, 1), ml_dtypes.bfloat16),
    }

    in_maps = []
    for c in range(NCORES):
        h0 = c * HPC
        wqaT = np.ascontiguousarray(Wqa[c * QLC:(c + 1) * QLC].T)   # [5120, 192]
        qbt = np.empty((PAIRS, QL, 384), ml_dtypes.bfloat16)
        abT = np.empty((PAIRS, KL, 4 * VD), ml_dtypes.bfloat16)
        for p in range(PAIRS):
            ha, hb = h0 + 2 * p, h0 + 2 * p + 1
            qbt[p, :, 0:128] = qb3[ha, :NOPE].T
            qbt[p, :, 128:256] = qb3[hb, :NOPE].T
            qbt[p, :, 256:320] = qb3[ha, NOPE:][perm].T
            qbt[p, :, 320:384] = qb3[hb, NOPE:][perm].T
            abT[p, :, 0:128] = kvb[ha, :NOPE].T      # q_absorb.T head a
            abT[p, :, 128:256] = kvb[hb, :NOPE].T    # q_absorb.T head b
            abT[p, :, 256:384] = kvb[ha, NOPE:].T    # out_absorb.T head a
            abT[p, :, 384:512] = kvb[hb, NOPE:].T    # out_absorb.T head b
        woT = np.ascontiguousarray(
            Wo[:, h0 * VD:(h0 + HPC) * VD].T
        ).astype(ml_dtypes.bfloat16)                                # [2048, 5120]
        in_maps.append({
            "hsqT": hsqT,
            "wqaT": wqaT,
            "qbt": qbt,
            "abT": abT,
            "ckv5": ckv5,
            "woT": woT,
        })
    return in_maps, consts


def _build_program(consts):
    nc = bacc.Bacc("TRN2", num_devices=NCORES)

    hsqT = nc.dram_tensor("hsqT", [HID, QLEN], F32R, kind="ExternalInput")
    wqaT = nc.dram_tensor("wqaT", [HID, QLC], F32R, kind="ExternalInput")
    qbt = nc.dram_tensor("qbt", [PAIRS, QL, 384], BF16, kind="ExternalInput")
    abT = nc.dram_tensor("abT", [PAIRS, KL, 4 * VD], BF16, kind="ExternalInput")
    ckv5 = nc.dram_tensor("ckv5", [640, KVLEN], BF16, kind="ExternalInput")
    woT = nc.dram_tensor("woT", [HPC * VD, HID], BF16, kind="ExternalInput")
    out_sh = nc.dram_tensor("out_shard", [TSH, HID], F32, kind="ExternalOutput")

    cos2_d = nc.inline_tensor(consts["cos2"], "cos2")
    sin2_d = nc.inline_tensor(consts["sin2"], "sin2")
    psignT_d = nc.inline_tensor(consts["psignT"], "psignT")
    onesf_d = nc.inline_tensor(consts["onesf"], "onesf")
    onesb_d = nc.inline_tensor(consts["onesb"], "onesb")

    # collective bounce buffers (internal DRAM)
    ag_in = nc.dram_tensor("ag_in", [QLC, QLEN], BF16)
    ar_in = nc.dram_tensor("ar_in", [1, QLEN], F32)
    ar_out = nc.dram_tensor("ar_out", [1, QLEN], F32)
    ag_out = nc.dram_tensor("ag_out", [QL, QLEN], BF16, addr_space="Shared")
    rs_in = nc.dram_tensor("rs_in", [QLEN, HID], F32)
    rs_out = nc.dram_tensor("rs_out", [TSH, HID], F32)
    RG = [list(range(NCORES))]

    with tile.TileContext(nc, num_cores=NCORES) as tc:
        with (
            tc.tile_pool(name="const", bufs=1) as constp,
            tc.tile_pool(name="ckv", bufs=1) as ckvp,
            tc.tile_pool(name="qts", bufs=1) as qtsp,
            tc.tile_pool(name="o16", bufs=1) as o16p,
            tc.tile_pool(name="vdec", bufs=3) as vp,
            tc.tile_pool(name="keff", bufs=6) as kp,
            tc.tile_pool(name="abt", bufs=3) as abp,
            tc.tile_pool(name="psV", bufs=1, space="PSUM") as psV,
            tc.tile_pool(name="psP", bufs=2, space="PSUM") as psP,
        ):
            cos2_s = constp.tile([128, QLEN], F32, tag="cos2")
            sin2_s = constp.tile([128, QLEN], F32, tag="sin2")
            psign_s = constp.tile([128, 128], BF16, tag="psign")
            onesf_s = constp.tile([128, 128], F32, tag="onesf")
            onesb_s = constp.tile([128, 1], BF16, tag="onesb")
            nc.sync.dma_start(cos2_s[:], cos2_d[:, :])
            nc.sync.dma_start(sin2_s[:], sin2_d[:, :])
            nc.sync.dma_start(psign_s[:], psignT_d[:, :])
            nc.sync.dma_start(onesf_s[:], onesf_d[:, :])
            nc.sync.dma_start(onesb_s[:], onesb_d[:, :])
            onesf_r = constp.tile([128, 128], F32R, tag="onesf_r")
            nc.vector.tensor_copy(onesf_r[:], onesf_s[:])

            ckv_s = ckvp.tile([128, 5, KVLEN], BF16)  # 4 c-chunks + [kpe;kpe]
            nc.sync.dma_start(
                ckv_s[:], ckv5[:, :].rearrange("(c p) k -> p c k", p=128)
            )

            qts = qtsp.tile([128, NJC, QLEN], BF16)      # scaled q.T
            o16 = o16p.tile([128, HPC, QLEN], BF16)      # normalized per-head o.T

            ab_tiles = {}

            def load_abt(p):
                t = abp.tile([128, NCC, 4 * VD], BF16, tag="abt")
                nc.sync.dma_start(
                    t[:], abT[p].rearrange("(c p) f -> p c f", p=128)
                )
                return t

            def v_decomp(p, ab_s):
                """Decompress V for pair p -> v tile [128k, kc, 2*VD] bf16."""
                v_s = vp.tile([128, NKC, 2 * VD], BF16, tag="v")
                for kc in range(NKC):
                    vps = psV.tile([128, 2 * VD], F32, tag="vps")
                    for ci in range(NCC):
                        nc.tensor.matmul(
                            vps[:],
                            ckv_s[:, ci, kc * 128:(kc + 1) * 128],
                            ab_s[:, ci, 256:512],
                            start=(ci == 0), stop=(ci == NCC - 1),
                        )
                    nc.vector.tensor_copy(v_s[:, kc, :], vps[:])
                return v_s

            def k_eff(hh, ab_s):
                """Decompressed nope-keys for one head: [128d, ks, 512k] bf16."""
                k_s = kp.tile([128, 4, QLEN], BF16, tag="keff")
                for ks in range(4):
                    kq = psP.tile([128, QLEN], F32, tag="pp")
                    for ci in range(NCC):
                        nc.tensor.matmul(
                            kq[:],
                            ab_s[:, ci, 128 * hh:128 * (hh + 1)],
                            ckv_s[:, ci, ks * 512:(ks + 1) * 512],
                            start=(ci == 0), stop=(ci == NCC - 1),
                        )
                    nc.scalar.copy(k_s[:, ks, :], kq[:])
                return k_s

            # ---------------- stage A: q = hsq @ Wqa.T (sharded) -------------
            with (
                tc.tile_pool(name="stgA", bufs=3) as sap,
                tc.tile_pool(name="psA", bufs=1, space="PSUM") as psA,
            ):
                qra = psA.tile([128, QLEN], F32, tag="qra")
                qrb = psA.tile([64, QLEN], F32, tag="qrb")
                for kc in range(NHID):
                    ht = sap.tile([128, QLEN], F32R, tag="ht")
                    nc.sync.dma_start(ht[:], hsqT[kc * 128:(kc + 1) * 128, :])
                    wt = sap.tile([128, QLC], F32R, tag="wt")
                    nc.sync.dma_start(wt[:], wqaT[kc * 128:(kc + 1) * 128, :])
                    nc.tensor.matmul(
                        qra[:], wt[:, 0:128], ht[:],
                        start=(kc == 0), stop=(kc == NHID - 1),
                    )
                    nc.tensor.matmul(
                        qrb[:], wt[:, 128:QLC], ht[:],
                        start=(kc == 0), stop=(kc == NHID - 1),
                    )
                sqa = sap.tile([128, QLEN], BF16, tag="sqa")
                sqb = sap.tile([64, QLEN], BF16, tag="sqb")
                nc.scalar.copy(sqa[:], qra[:])
                nc.scalar.copy(sqb[:], qrb[:])
                nc.sync.dma_start(ag_in[0:128, :], sqa[:])
                nc.sync.dma_start(ag_in[128:QLC, :], sqb[:])
                # local partial sum-of-squares over this core's 192 q rows
                # (from the fp32 psum accumulators, pre-rounding)
                sq2a = sap.tile([128, QLEN], F32R, tag="sq2a")
                sq2b = sap.tile([64, QLEN], F32R, tag="sq2b")
                nc.vector.tensor_mul(sq2a[:], qra[:], sqa[:])
                nc.vector.tensor_mul(sq2b[:], qrb[:], sqb[:])
                ssqp = psA.tile([1, QLEN], F32, tag="ssqp")
                nc.tensor.matmul(
                    ssqp[:], onesf_r[:, 0:1], sq2a[:], start=True, stop=False
                )
                nc.tensor.matmul(
                    ssqp[:], onesf_r[0:64, 0:1], sq2b[:], start=False, stop=True
                )
                ssq_s = sap.tile([1, QLEN], F32, tag="ssq_s")
                nc.scalar.copy(ssq_s[:], ssqp[:])
                nc.sync.dma_start(ar_in[:, :], ssq_s[:])

            nc.gpsimd.collective_compute(
                "AllGather", mybir.AluOpType.bypass,
                replica_groups=RG,
                ins=[ag_in[:, :].opt()], outs=[ag_out[:, :].opt()],
            )
            nc.gpsimd.collective_compute(
                "AllReduce", mybir.AluOpType.add,
                replica_groups=RG,
                ins=[ar_in[:, :].opt()], outs=[ar_out[:, :].opt()],
            )

            # q-independent work fills the collective wait: V and K
            # decompression for the first pairs
            v_tiles = {}
            keff_tiles = {}
            for p in (0, 1, 2):
                ab_tiles[p] = load_abt(p)
            for p in (0, 1):
                v_tiles[p] = v_decomp(p, ab_tiles[p])
            for h in range(6):
                keff_tiles[h] = k_eff(h % 2, ab_tiles[h // 2])

            # -------- stage B: gathered q + rms scale vector --------
            # qts holds the UNscaled bf16 q.T; the per-token rms scale is
            # applied to the (much smaller) per-pair projections instead.
            nc.sync.dma_start(
                qts[:], ag_out[:, :].rearrange("(c p) t -> p c t", p=128)
            )
            with tc.tile_pool(name="stgBs", bufs=1) as sbs:
                # r_scaled = (1/sqrt(192)) * rsqrt(ssq/1536 + eps)
                #          = 1 / sqrt(ssq*0.125 + 192*eps)
                ssqf = sbs.tile([1, QLEN], F32, tag="ssqf")
                nc.sync.dma_start(ssqf[:], ar_out[:, :])
                eps_s = sbs.tile([1, 1], F32, tag="eps")
                nc.gpsimd.memset(eps_s[:], QHD * EPS)
                sqv = sbs.tile([1, QLEN], F32, tag="sqv")
                nc.scalar.activation(
                    sqv[:], ssqf[:], AF.Sqrt, scale=QHD / QL, bias=eps_s[:]
                )
                rsc = sbs.tile([1, QLEN], F32, tag="rsc")
                nc.vector.reciprocal(rsc[:], sqv[:])
                bcs = constp.tile([128, QLEN], F32, tag="bcs")
                nc.gpsimd.partition_broadcast(bcs[:], rsc[:])

            # ---------------- per-pair attention ----------------
            with (
                tc.tile_pool(name="qbt", bufs=2) as qbtp,
                tc.tile_pool(name="hsb", bufs=2) as hsb,     # per-pair sbuf
                tc.tile_pool(name="expp", bufs=4) as expp,
                tc.tile_pool(name="sml", bufs=2) as sml,
                tc.tile_pool(name="psL", bufs=2, space="PSUM") as psL,
                tc.tile_pool(name="psO", bufs=2, space="PSUM") as psO,
                tc.tile_pool(name="psS", bufs=1, space="PSUM") as psS,
            ):
                def load_qbt(p):
                    t = qbtp.tile([128, NJC, 384], BF16, tag="qbt")
                    nc.sync.dma_start(
                        t[:], qbt[p].rearrange("(c p) f -> p c f", p=128)
                    )
                    return t

                def pair_qproj(qbt_s):
                    """-> (qn_sb[2], roped) for the pair."""
                    qn_sb = []
                    for part in range(2):   # nope head a, nope head b
                        qn_ps = psP.tile([128, QLEN], F32, tag="pp")
                        for jc in range(NJC):
                            nc.tensor.matmul(
                                qn_ps[:],
                                qbt_s[:, jc, 128 * part:128 * (part + 1)],
                                qts[:, jc, :],
                                start=(jc == 0), stop=(jc == NJC - 1),
                            )
                        s = hsb.tile([128, QLEN], BF16, tag="qn")
                        nc.vector.tensor_mul(s[:], qn_ps[:], bcs[:])
                        qn_sb.append(s)
                    pe_ps = psP.tile([128, QLEN], F32, tag="pp")
                    for jc in range(NJC):
                        nc.tensor.matmul(
                            pe_ps[:], qbt_s[:, jc, 256:384],
                            qts[:, jc, :],
                            start=(jc == 0), stop=(jc == NJC - 1),
                        )
                    pe_sb = hsb.tile([128, QLEN], BF16, tag="pe")
                    nc.vector.tensor_mul(pe_sb[:], pe_ps[:], bcs[:])
                    rot_ps = psP.tile([128, QLEN], F32, tag="pp")
                    nc.tensor.matmul(
                        rot_ps[:], psign_s[:], pe_sb[:],
                        start=True, stop=True,
                    )
                    tmp1 = hsb.tile([128, QLEN], F32, tag="tmp1")
                    nc.vector.tensor_mul(tmp1[:], pe_sb[:], cos2_s[:])
                    tmp2 = hsb.tile([128, QLEN], F32, tag="tmp2")
                    nc.vector.tensor_mul(tmp2[:], rot_ps[:], sin2_s[:])
                    roped = hsb.tile([128, QLEN], BF16, tag="roped")
                    nc.vector.tensor_add(roped[:], tmp1[:], tmp2[:])
                    return qn_sb, roped

                def pair_attn(p, qn_sb, roped, v_s, keffs):
                    """Merged kc loop over both heads of the pair.

                    Both heads' softmax row-sums accumulate in ONE psum bank
                    at disjoint partitions (0 and 64) so every downstream
                    consumer stays partition-aligned."""
                    oT = [psO.tile([128, QLEN], F32, tag="ot", name=f"oT{p}_{i}")
                          for i in range(2)]
                    ssum2 = psS.tile([65, QLEN], F32, tag="ssum")
                    ssum = [ssum2[0:1, :], ssum2[64:65, :]]
                    for kc in range(NKC):
                        lg = [psL.tile([128, QLEN], F32, tag="lg",
                                       name=f"lg{p}_{kc}_{i}") for i in range(2)]
                        for hh in range(2):
                            nc.tensor.matmul(
                                lg[hh][:],
                                keffs[hh][:, kc // 4,
                                          (kc % 4) * 128:(kc % 4 + 1) * 128],
                                qn_sb[hh][:],
                                start=True, stop=False,
                            )
                        # the two K=64 rope matmuls target disjoint PE row
                        # groups (0:64 / 64:128) and run concurrently
                        for hh in range(2):
                            nc.tensor.matmul(
                                lg[hh][:],
                                ckv_s[:, 4, kc * 128:(kc + 1) * 128][
                                    64 * hh:64 * (hh + 1), :],
                                roped[64 * hh:64 * (hh + 1), :],
                                start=False, stop=True,
                            )
                        for hh in range(2):
                            ex = expp.tile([128, QLEN], BF16, tag="ex")
                            nc.scalar.activation(ex[:], lg[hh][:], AF.Exp)
                            nc.tensor.matmul(
                                oT[hh][:],
                                v_s[:, kc, VD * hh:VD * (hh + 1)],
                                ex[:],
                                start=(kc == 0), stop=(kc == NKC - 1),
                            )
                            nc.tensor.matmul(
                                ssum[hh], onesb_s[:], ex[:],
                                start=(kc == 0), stop=(kc == NKC - 1),
                                skip_group_check=True,
                            )
                    def finalize(inv2):
                        for hh in range(2):
                            row = slice(64 * hh, 64 * hh + 1)
                            nc.vector.reciprocal(inv2[row, :], ssum[hh])
                            if hh == 0:
                                src_row = inv2[0:1, :]
                            else:
                                # partition_broadcast ucode always reads the
                                # physical partition 0 — move the row there
                                inv_b0 = sml.tile([1, QLEN], F32, tag="inv_b0")
                                nc.gpsimd.dma_start(inv_b0[:], inv2[64:65, :])
                                src_row = inv_b0[:]
                            binv = sml.tile([128, QLEN], F32, tag="binv")
                            nc.gpsimd.partition_broadcast(binv[:], src_row)
                            nc.vector.tensor_mul(
                                o16[:, 2 * p + hh, :], oT[hh][:], binv[:]
                            )
                    return finalize

                qbt_tiles = {0: load_qbt(0)}
                for p in range(PAIRS):
                    qbt_s = qbt_tiles.pop(p)
                    if p + 1 < PAIRS:
                        qbt_tiles[p + 1] = load_qbt(p + 1)
                    qn_sb, roped = pair_qproj(qbt_s)
                    v_s = v_tiles.pop(p)
                    keffs = [keff_tiles.pop(2 * p), keff_tiles.pop(2 * p + 1)]
                    finalize = pair_attn(p, qn_sb, roped, v_s, keffs)
                    # prefetch decompression for later pairs (emitted before
                    # the norm chain so its casts sit ahead of the slow
                    # reciprocals in the DVE queue)
                    ab_tiles.pop(p, None)
                    if p + 3 < PAIRS:
                        ab_tiles[p + 3] = load_abt(p + 3)
                    if p + 2 < PAIRS:
                        v_tiles[p + 2] = v_decomp(p + 2, ab_tiles[p + 2])
                    if p + 3 < PAIRS:
                        keff_tiles[2 * p + 6] = k_eff(0, ab_tiles[p + 3])
                        keff_tiles[2 * p + 7] = k_eff(1, ab_tiles[p + 3])
                    finalize(sml.tile([65, QLEN], F32, tag="inv", name=f"inv{p}"))

            # ---------------- output projection + ReduceScatter --------------
            with (
                tc.tile_pool(name="wo", bufs=2) as wop,
                tc.tile_pool(name="osb", bufs=3) as osb,
                tc.tile_pool(name="psW", bufs=2, space="PSUM") as psW,
            ):
                for ds in range(NDS):
                    w16 = wop.tile([128, HPC, 512], BF16, tag="w16")
                    nc.sync.dma_start(
                        w16[:],
                        woT[:, :].rearrange("(g p) d -> p g d", p=128)[
                            :, :, ds * 512:(ds + 1) * 512],
                    )
                    for tc4 in range(4):
                        acc = psW.tile([128, 512], F32, tag="acc")
                        for g in range(HPC):
                            nc.tensor.matmul(
                                acc[:],
                                o16[:, g, tc4 * 128:(tc4 + 1) * 128],
                                w16[:, g, :],
                                start=(g == 0), stop=(g == HPC - 1),
                            )
                        ot = osb.tile([128, 512], F32, tag="ot")
                        nc.vector.tensor_copy(ot[:], acc[:])
                        nc.sync.dma_start(
                            rs_in[tc4 * 128:(tc4 + 1) * 128,
                                  ds * 512:(ds + 1) * 512],
                            ot[:],
                        )

            nc.gpsimd.collective_compute(
                "ReduceScatter", mybir.AluOpType.add,
                replica_groups=RG,
                ins=[rs_in[:, :].opt()], outs=[rs_out[:, :].opt()],
            )
            nc.gpsimd.dma_start(out_sh[:, :], rs_out[:, :])

    nc.compile()
    return nc


_CACHE = {}


def _get_program(consts):
    key = (consts["cos2"].tobytes(), consts["sin2"].tobytes())
    if key not in _CACHE:
        _CACHE[key] = _build_program(consts)
    return _CACHE[key]


def _run(inputs, **kwargs):
    in_maps, consts = _host_prepare(inputs)
    nc = _get_program(consts)
    res = run_bass_kernel_spmd(nc, in_maps, core_ids=list(range(NCORES)), **kwargs)
    shards = [res.results[c]["out_shard"] for c in range(NCORES)]
    out = np.concatenate(shards, axis=0)[None].astype(np.float32)
    return out, res


def kernel(**inputs) -> np.ndarray:
    return _run(inputs)[0]


# revision 20
# speedup vs baseline: 1.4697x; 1.0751x over previous
"""DeepSeek-V2 MLA attention (weight-absorbed) on 8 Trainium2 NeuronCores.

Sharding: tensor-parallel over the 128 heads (16 heads/core).  The q
LoRA projection (hidden @ Wqa.T) is sharded over the Q_LORA output dim
and AllGathered; the per-head attention runs fully local; the output
projection partials are summed with a ReduceScatter over the token axis
and the 8 shards are concatenated on the host.

Math restructuring vs the reference (exactly associativity-equivalent):
  - q_nope = (q @ qb_nope.T) @ q_absorb          (factor through the 128-dim)
  - o      = softmax(l) @ (ckv @ out_absorb.T)   (decompress V, 128-dim)
  - out    = concat_h(o_h) @ Wo.T                (plain o_proj)
  - rmsnorm's per-token scale and 1/sqrt(192) are folded into q;
    qa_ln_w is folded into Wqb; the RoPE interleave permutation is
    folded into the rope rows of Wqb; softmax skips the max-subtraction
    (logits are O(3) for this problem) and normalizes o after PV.
"""

import math
import numpy as np
import ml_dtypes

import concourse.bass as bass
import concourse.bacc as bacc
import concourse.mybir as mybir
import concourse.tile as tile
from concourse.bass_utils import run_bass_kernel_spmd

F32 = mybir.dt.float32
F32R = mybir.dt.float32r
BF16 = mybir.dt.bfloat16
AF = mybir.ActivationFunctionType

H, QL, KL, ROPE, NOPE, VD, HID = 128, 1536, 512, 64, 128, 128, 5120
QHD = NOPE + ROPE  # 192
QLEN, KVLEN = 512, 2048
NCORES = 8
HPC = H // NCORES          # 16 heads per core
PAIRS = HPC // 2           # 8 pairs per core
QLC = QL // NCORES         # 192 q-lora rows per core
TSH = QLEN // NCORES       # 64 token rows per output shard
NKC = KVLEN // 128         # 16 kv chunks
NJC = QL // 128            # 12 q-lora chunks
NCC = KL // 128            # 4 compressed-kv chunks
NHID = HID // 128          # 40 hidden chunks
NDS = HID // 512           # 10 output dim slices
EPS = 1e-6





def _host_prepare(inputs):
    """Full inputs -> (list of per-core input dicts, const arrays)."""
    hsq = np.asarray(inputs["hidden_states_q"], np.float32)[0]      # [512, 5120]
    pos = np.asarray(inputs["q_position_ids"])[0]                   # [512]
    ckv_full = np.asarray(inputs["compressed_kv"], np.float32)[0]   # [2048, 576]
    Wqa = np.asarray(inputs["Wqa"], np.float32)                     # [1536, 5120]
    w_ln = np.asarray(inputs["qa_ln_w"], np.float32)                # [1536]
    Wqb = np.asarray(inputs["Wqb"], np.float32)                     # [24576, 1536]
    Wkvb = np.asarray(inputs["Wkvb"], np.float32)                   # [32768, 512]
    Wo = np.asarray(inputs["Wo"], np.float32)                       # [5120, 16384]

    hsqT = np.ascontiguousarray(hsq.T)                              # [5120, 512]
    ckvT = np.ascontiguousarray(ckv_full.T)                         # [576, 2048]
    kpeT = ckvT[KL:]                                                # [64, 2048]
    # c chunks + k_pe duplicated twice (so both heads of a pair can use
    # partition-aligned lhsT slices at base 0 / 64)
    ckv5 = np.concatenate([ckvT[:KL], kpeT, kpeT], axis=0).astype(
        ml_dtypes.bfloat16)                                         # [640, 2048]

    Wqb_w = Wqb * w_ln[None, :]
    qb3 = Wqb_w.reshape(H, QHD, QL)
    kvb = Wkvb.reshape(H, NOPE + VD, KL)
    perm = np.concatenate([np.arange(0, ROPE, 2), np.arange(1, ROPE, 2)])

    # rope tables in half-split layout, [d, t]; doubled over the pair axis
    inv_freq = 1.0 / (10000.0 ** (np.arange(0, ROPE, 2, dtype=np.float64) / ROPE))
    fr = np.outer(pos.astype(np.float64), inv_freq)                 # [512, 32]
    emb = np.concatenate([fr, fr], axis=-1)                         # [512, 64]
    cosT = np.cos(emb).T.astype(np.float32)                         # [64, 512]
    sinT = np.sin(emb).T.astype(np.float32)
    cos2 = np.ascontiguousarray(np.concatenate([cosT, cosT], axis=0))  # [128, 512]
    sin2 = np.ascontiguousarray(np.concatenate([sinT, sinT], axis=0))

    # rot = blockdiag(P, P) @ q'   with  rot_h = [-q'[32:], q'[:32]]
    P64 = np.zeros((ROPE, ROPE), np.float32)
    P64[np.arange(32), np.arange(32) + 32] = -1.0
    P64[np.arange(32, 64), np.arange(32)] = 1.0
    psign = np.zeros((128, 128), np.float32)
    psign[:64, :64] = P64
    psign[64:, 64:] = P64
    psignT = np.ascontiguousarray(psign.T)

    consts = {
        "cos2": cos2,
        "sin2": sin2,
        "psignT": psignT.astype(ml_dtypes.bfloat16),
        "onesf": np.ones((128, 128), np.float32),
        "onesb": np.ones((128, 1), ml_dtypes.bfloat16),
    }

    in_maps = []
    for c in range(NCORES):
        h0 = c * HPC
        wqaT = np.ascontiguousarray(Wqa[c * QLC:(c + 1) * QLC].T)   # [5120, 192]
        qbt = np.empty((PAIRS, QL, 384), ml_dtypes.bfloat16)
        abT = np.empty((PAIRS, KL, 4 * VD), ml_dtypes.bfloat16)
        for p in range(PAIRS):
            ha, hb = h0 + 2 * p, h0 + 2 * p + 1
            qbt[p, :, 0:128] = qb3[ha, :NOPE].T
            qbt[p, :, 128:256] = qb3[hb, :NOPE].T
            qbt[p, :, 256:320] = qb3[ha, NOPE:][perm].T
            qbt[p, :, 320:384] = qb3[hb, NOPE:][perm].T
            abT[p, :, 0:128] = kvb[ha, :NOPE].T      # q_absorb.T head a
            abT[p, :, 128:256] = kvb[hb, :NOPE].T    # q_absorb.T head b
            abT[p, :, 256:384] = kvb[ha, NOPE:].T    # out_absorb.T head a
            abT[p, :, 384:512] = kvb[hb, NOPE:].T    # out_absorb.T head b
        woT = np.ascontiguousarray(
            Wo[:, h0 * VD:(h0 + HPC) * VD].T
        ).astype(ml_dtypes.bfloat16)                                # [2048, 5120]
        in_maps.append({
            "hsqT": hsqT,
            "wqaT": wqaT,
            "qbt": qbt,
            "abT": abT,
            "ckv5": ckv5,
            "woT": woT,
        })
    return in_maps, consts


def _build_program(consts):
    nc = bacc.Bacc("TRN2", num_devices=NCORES)

    hsqT = nc.dram_tensor("hsqT", [HID, QLEN], F32R, kind="ExternalInput")
    wqaT = nc.dram_tensor("wqaT", [HID, QLC], F32R, kind="ExternalInput")
    qbt = nc.dram_tensor("qbt", [PAIRS, QL, 384], BF16, kind="ExternalInput")
    abT = nc.dram_tensor("abT", [PAIRS, KL, 4 * VD], BF16, kind="ExternalInput")
    ckv5 = nc.dram_tensor("ckv5", [640, KVLEN], BF16, kind="ExternalInput")
    woT = nc.dram_tensor("woT", [HPC * VD, HID], BF16, kind="ExternalInput")
    out_sh = nc.dram_tensor("out_shard", [TSH, HID], F32, kind="ExternalOutput")

    cos2_d = nc.inline_tensor(consts["cos2"], "cos2")
    sin2_d = nc.inline_tensor(consts["sin2"], "sin2")
    psignT_d = nc.inline_tensor(consts["psignT"], "psignT")
    onesf_d = nc.inline_tensor(consts["onesf"], "onesf")
    onesb_d = nc.inline_tensor(consts["onesb"], "onesb")

    # collective bounce buffers (internal DRAM)
    ag_in = nc.dram_tensor("ag_in", [QLC, QLEN], BF16)
    ar_in = nc.dram_tensor("ar_in", [1, QLEN], F32)
    ar_out = nc.dram_tensor("ar_out", [1, QLEN], F32)
    ag_out = nc.dram_tensor("ag_out", [QL, QLEN], BF16, addr_space="Shared")
    rs_in_a = nc.dram_tensor("rs_in_a", [QLEN, HID // 2], BF16)
    rs_in_b = nc.dram_tensor("rs_in_b", [QLEN, HID // 2], BF16)
    rs_out_a = nc.dram_tensor("rs_out_a", [TSH, HID // 2], BF16)
    rs_out_b = nc.dram_tensor("rs_out_b", [TSH, HID // 2], BF16)
    RG = [list(range(NCORES))]

    with tile.TileContext(nc, num_cores=NCORES) as tc:
        with (
            tc.tile_pool(name="const", bufs=1) as constp,
            tc.tile_pool(name="ckv", bufs=1) as ckvp,
            tc.tile_pool(name="qts", bufs=1) as qtsp,
            tc.tile_pool(name="o16", bufs=1) as o16p,
            tc.tile_pool(name="vdec", bufs=4) as vp,
            tc.tile_pool(name="keff", bufs=8) as kp,
            tc.tile_pool(name="abt", bufs=3) as abp,
            tc.tile_pool(name="psV", bufs=1, space="PSUM") as psV,
            tc.tile_pool(name="psP", bufs=2, space="PSUM") as psP,
        ):
            cos2_s = constp.tile([128, QLEN], F32, tag="cos2")
            sin2_s = constp.tile([128, QLEN], F32, tag="sin2")
            psign_s = constp.tile([128, 128], BF16, tag="psign")
            onesf_s = constp.tile([128, 128], F32, tag="onesf")
            onesb_s = constp.tile([128, 1], BF16, tag="onesb")
            nc.sync.dma_start(cos2_s[:], cos2_d[:, :])
            nc.sync.dma_start(sin2_s[:], sin2_d[:, :])
            nc.sync.dma_start(psign_s[:], psignT_d[:, :])
            nc.sync.dma_start(onesf_s[:], onesf_d[:, :])
            nc.sync.dma_start(onesb_s[:], onesb_d[:, :])
            onesf_r = constp.tile([128, 128], F32R, tag="onesf_r")
            nc.vector.tensor_copy(onesf_r[:], onesf_s[:])

            ckv_s = ckvp.tile([128, 5, KVLEN], BF16)  # 4 c-chunks + [kpe;kpe]
            nc.sync.dma_start(
                ckv_s[:], ckv5[:, :].rearrange("(c p) k -> p c k", p=128)
            )

            qts = qtsp.tile([128, NJC, QLEN], BF16)      # scaled q.T
            o16 = o16p.tile([128, HPC, QLEN], BF16)      # normalized per-head o.T

            ab_tiles = {}

            def load_abt(p):
                t = abp.tile([128, NCC, 4 * VD], BF16, tag="abt")
                nc.sync.dma_start(
                    t[:], abT[p].rearrange("(c p) f -> p c f", p=128)
                )
                return t

            def v_decomp(p, ab_s):
                """Decompress V for pair p -> v tile [128k, kc, 2*VD] bf16."""
                v_s = vp.tile([128, NKC, 2 * VD], BF16, tag="v")
                for kc in range(NKC):
                    vps = psV.tile([128, 2 * VD], F32, tag="vps")
                    for ci in range(NCC):
                        nc.tensor.matmul(
                            vps[:],
                            ckv_s[:, ci, kc * 128:(kc + 1) * 128],
                            ab_s[:, ci, 256:512],
                            start=(ci == 0), stop=(ci == NCC - 1),
                        )
                    nc.vector.tensor_copy(v_s[:, kc, :], vps[:])
                return v_s

            def k_eff(hh, ab_s):
                """Decompressed nope-keys for one head: [128d, ks, 512k] bf16."""
                k_s = kp.tile([128, 4, QLEN], BF16, tag="keff")
                for ks in range(4):
                    kq = psP.tile([128, QLEN], F32, tag="pp")
                    for ci in range(NCC):
                        nc.tensor.matmul(
                            kq[:],
                            ab_s[:, ci, 128 * hh:128 * (hh + 1)],
                            ckv_s[:, ci, ks * 512:(ks + 1) * 512],
                            start=(ci == 0), stop=(ci == NCC - 1),
                        )
                    nc.scalar.copy(k_s[:, ks, :], kq[:])
                return k_s

            # ---------------- stage A: q = hsq @ Wqa.T (sharded) -------------
            with (
                tc.tile_pool(name="stgA", bufs=3) as sap,
                tc.tile_pool(name="psA", bufs=1, space="PSUM") as psA,
            ):
                qra = psA.tile([128, QLEN], F32, tag="qra")
                qrb = psA.tile([64, QLEN], F32, tag="qrb")
                for kc in range(NHID):
                    ht = sap.tile([128, QLEN], F32R, tag="ht")
                    nc.sync.dma_start(ht[:], hsqT[kc * 128:(kc + 1) * 128, :])
                    wt = sap.tile([128, QLC], F32R, tag="wt")
                    nc.sync.dma_start(wt[:], wqaT[kc * 128:(kc + 1) * 128, :])
                    nc.tensor.matmul(
                        qra[:], wt[:, 0:128], ht[:],
                        start=(kc == 0), stop=(kc == NHID - 1),
                    )
                    nc.tensor.matmul(
                        qrb[:], wt[:, 128:QLC], ht[:],
                        start=(kc == 0), stop=(kc == NHID - 1),
                    )
                sqa = sap.tile([128, QLEN], BF16, tag="sqa")
                sqb = sap.tile([64, QLEN], BF16, tag="sqb")
                nc.scalar.copy(sqa[:], qra[:])
                nc.scalar.copy(sqb[:], qrb[:])
                nc.sync.dma_start(ag_in[0:128, :], sqa[:])
                nc.sync.dma_start(ag_in[128:QLC, :], sqb[:])
                # local partial sum-of-squares over this core's 192 q rows
                # (from the fp32 psum accumulators, pre-rounding)
                sq2a = sap.tile([128, QLEN], F32R, tag="sq2a")
                sq2b = sap.tile([64, QLEN], F32R, tag="sq2b")
                nc.vector.tensor_mul(sq2a[:], qra[:], sqa[:])
                nc.vector.tensor_mul(sq2b[:], qrb[:], sqb[:])
                ssqp = psA.tile([1, QLEN], F32, tag="ssqp")
                nc.tensor.matmul(
                    ssqp[:], onesf_r[:, 0:1], sq2a[:], start=True, stop=False
                )
                nc.tensor.matmul(
                    ssqp[:], onesf_r[0:64, 0:1], sq2b[:], start=False, stop=True
                )
                ssq_s = sap.tile([1, QLEN], F32, tag="ssq_s")
                nc.scalar.copy(ssq_s[:], ssqp[:])
                nc.sync.dma_start(ar_in[:, :], ssq_s[:])

            nc.gpsimd.collective_compute(
                "AllGather", mybir.AluOpType.bypass,
                replica_groups=RG,
                ins=[ag_in[:, :].opt()], outs=[ag_out[:, :].opt()],
            )
            nc.gpsimd.collective_compute(
                "AllReduce", mybir.AluOpType.add,
                replica_groups=RG,
                ins=[ar_in[:, :].opt()], outs=[ar_out[:, :].opt()],
            )

            # q-independent work fills the collective wait: V and K
            # decompression for the first pairs
            v_tiles = {}
            keff_tiles = {}
            for p in (0, 1, 2, 3):
                ab_tiles[p] = load_abt(p)
            for p in (0, 1, 2):
                v_tiles[p] = v_decomp(p, ab_tiles[p])
            for h in range(6):
                keff_tiles[h] = k_eff(h % 2, ab_tiles[h // 2])

            # -------- stage B: gathered q + rms scale vector --------
            # qts holds the UNscaled bf16 q.T; the per-token rms scale is
            # applied to the (much smaller) per-pair projections instead.
            nc.sync.dma_start(
                qts[:], ag_out[:, :].rearrange("(c p) t -> p c t", p=128)
            )
            with tc.tile_pool(name="stgBs", bufs=1) as sbs:
                # r_scaled = (1/sqrt(192)) * rsqrt(ssq/1536 + eps)
                #          = 1 / sqrt(ssq*0.125 + 192*eps)
                ssqf = sbs.tile([1, QLEN], F32, tag="ssqf")
                nc.sync.dma_start(ssqf[:], ar_out[:, :])
                eps_s = sbs.tile([1, 1], F32, tag="eps")
                nc.gpsimd.memset(eps_s[:], QHD * EPS)
                sqv = sbs.tile([1, QLEN], F32, tag="sqv")
                nc.scalar.activation(
                    sqv[:], ssqf[:], AF.Sqrt, scale=QHD / QL, bias=eps_s[:]
                )
                rsc = sbs.tile([1, QLEN], F32, tag="rsc")
                nc.vector.reciprocal(rsc[:], sqv[:])
                bcs = constp.tile([128, QLEN], F32, tag="bcs")
                nc.gpsimd.partition_broadcast(bcs[:], rsc[:])

            # ---------------- per-pair attention ----------------
            with (
                tc.tile_pool(name="qbt", bufs=2) as qbtp,
                tc.tile_pool(name="hsb", bufs=2) as hsb,     # per-pair sbuf
                tc.tile_pool(name="expp", bufs=4) as expp,
                tc.tile_pool(name="sml", bufs=2) as sml,
                tc.tile_pool(name="psL", bufs=2, space="PSUM") as psL,
                tc.tile_pool(name="psO", bufs=2, space="PSUM") as psO,
                tc.tile_pool(name="psS", bufs=1, space="PSUM") as psS,
            ):
                def load_qbt(p):
                    t = qbtp.tile([128, NJC, 384], BF16, tag="qbt")
                    nc.sync.dma_start(
                        t[:], qbt[p].rearrange("(c p) f -> p c f", p=128)
                    )
                    return t

                def pair_qproj(qbt_s):
                    """-> (qn_sb[2], roped) for the pair."""
                    qn_sb = []
                    for part in range(2):   # nope head a, nope head b
                        qn_ps = psP.tile([128, QLEN], F32, tag="pp")
                        for jc in range(NJC):
                            nc.tensor.matmul(
                                qn_ps[:],
                                qbt_s[:, jc, 128 * part:128 * (part + 1)],
                                qts[:, jc, :],
                                start=(jc == 0), stop=(jc == NJC - 1),
                            )
                        s = hsb.tile([128, QLEN], BF16, tag="qn")
                        nc.vector.tensor_mul(s[:], qn_ps[:], bcs[:])
                        qn_sb.append(s)
                    pe_ps = psP.tile([128, QLEN], F32, tag="pp")
                    for jc in range(NJC):
                        nc.tensor.matmul(
                            pe_ps[:], qbt_s[:, jc, 256:384],
                            qts[:, jc, :],
                            start=(jc == 0), stop=(jc == NJC - 1),
                        )
                    pe_sb = hsb.tile([128, QLEN], BF16, tag="pe")
                    nc.vector.tensor_mul(pe_sb[:], pe_ps[:], bcs[:])
                    rot_ps = psP.tile([128, QLEN], F32, tag="pp")
                    nc.tensor.matmul(
                        rot_ps[:], psign_s[:], pe_sb[:],
                        start=True, stop=True,
                    )
                    tmp1 = hsb.tile([128, QLEN], F32, tag="tmp1")
                    nc.vector.tensor_mul(tmp1[:], pe_sb[:], cos2_s[:])
                    tmp2 = hsb.tile([128, QLEN], F32, tag="tmp2")
                    nc.vector.tensor_mul(tmp2[:], rot_ps[:], sin2_s[:])
                    roped = hsb.tile([128, QLEN], BF16, tag="roped")
                    nc.vector.tensor_add(roped[:], tmp1[:], tmp2[:])
                    return qn_sb, roped

                def pair_attn(p, qn_sb, roped, v_s, keffs):
                    """Merged kc loop over both heads of the pair.

                    Both heads' softmax row-sums accumulate in ONE psum bank
                    at disjoint partitions (0 and 64) so every downstream
                    consumer stays partition-aligned."""
                    oT = [psO.tile([128, QLEN], F32, tag="ot", name=f"oT{p}_{i}")
                          for i in range(2)]
                    ssum2 = psS.tile([65, QLEN], F32, tag="ssum")
                    ssum = [ssum2[0:1, :], ssum2[64:65, :]]
                    for kc in range(NKC):
                        lg = [psL.tile([128, QLEN], F32, tag="lg",
                                       name=f"lg{p}_{kc}_{i}") for i in range(2)]
                        for hh in range(2):
                            nc.tensor.matmul(
                                lg[hh][:],
                                keffs[hh][:, kc // 4,
                                          (kc % 4) * 128:(kc % 4 + 1) * 128],
                                qn_sb[hh][:],
                                start=True, stop=False,
                            )
                        # the two K=64 rope matmuls target disjoint PE row
                        # groups (0:64 / 64:128) and run concurrently
                        for hh in range(2):
                            nc.tensor.matmul(
                                lg[hh][:],
                                ckv_s[:, 4, kc * 128:(kc + 1) * 128][
                                    64 * hh:64 * (hh + 1), :],
                                roped[64 * hh:64 * (hh + 1), :],
                                start=False, stop=True,
                            )
                        for hh in range(2):
                            ex = expp.tile([128, QLEN], BF16, tag="ex")
                            nc.scalar.activation(ex[:], lg[hh][:], AF.Exp)
                            nc.tensor.matmul(
                                oT[hh][:],
                                v_s[:, kc, VD * hh:VD * (hh + 1)],
                                ex[:],
                                start=(kc == 0), stop=(kc == NKC - 1),
                            )
                            nc.tensor.matmul(
                                ssum[hh], onesb_s[:], ex[:],
                                start=(kc == 0), stop=(kc == NKC - 1),
                                skip_group_check=True,
                            )
                    def finalize(inv2):
                        for hh in range(2):
                            row = slice(64 * hh, 64 * hh + 1)
                            nc.vector.reciprocal(inv2[row, :], ssum[hh])
                            if hh == 0:
                                src_row = inv2[0:1, :]
                            else:
                                # partition_broadcast ucode always reads the
                                # physical partition 0 — move the row there
                                inv_b0 = sml.tile([1, QLEN], F32, tag="inv_b0")
                                nc.gpsimd.dma_start(inv_b0[:], inv2[64:65, :])
                                src_row = inv_b0[:]
                            binv = sml.tile([128, QLEN], F32, tag="binv")
                            nc.gpsimd.partition_broadcast(binv[:], src_row)
                            nc.vector.tensor_mul(
                                o16[:, 2 * p + hh, :], oT[hh][:], binv[:]
                            )
                    return finalize

                qbt_tiles = {0: load_qbt(0)}
                for p in range(PAIRS):
                    qbt_s = qbt_tiles.pop(p)
                    if p + 1 < PAIRS:
                        qbt_tiles[p + 1] = load_qbt(p + 1)
                    qn_sb, roped = pair_qproj(qbt_s)
                    v_s = v_tiles.pop(p)
                    keffs = [keff_tiles.pop(2 * p), keff_tiles.pop(2 * p + 1)]
                    finalize = pair_attn(p, qn_sb, roped, v_s, keffs)
                    # prefetch decompression for later pairs (emitted before
                    # the norm chain so its casts sit ahead of the slow
                    # reciprocals in the DVE queue)
                    ab_tiles.pop(p, None)
                    if p + 4 < PAIRS:
                        ab_tiles[p + 4] = load_abt(p + 4)
                    if p + 3 < PAIRS:
                        v_tiles[p + 3] = v_decomp(p + 3, ab_tiles[p + 3])
                        keff_tiles[2 * p + 6] = k_eff(0, ab_tiles[p + 3])
                        keff_tiles[2 * p + 7] = k_eff(1, ab_tiles[p + 3])
                    finalize(sml.tile([65, QLEN], F32, tag="inv", name=f"inv{p}"))

            # ---------------- output projection + ReduceScatter --------------
            with (
                tc.tile_pool(name="wo", bufs=2) as wop,
                tc.tile_pool(name="osb", bufs=3) as osb,
                tc.tile_pool(name="psW", bufs=2, space="PSUM") as psW,
            ):
                for ds in range(NDS):
                    half, dsl = (rs_in_a, ds) if ds < 5 else (rs_in_b, ds - 5)
                    w16 = wop.tile([128, HPC, 512], BF16, tag="w16")
                    nc.sync.dma_start(
                        w16[:],
                        woT[:, :].rearrange("(g p) d -> p g d", p=128)[
                            :, :, ds * 512:(ds + 1) * 512],
                    )
                    for tc4 in range(4):
                        acc = psW.tile([128, 512], F32, tag="acc")
                        for g in range(HPC):
                            nc.tensor.matmul(
                                acc[:],
                                o16[:, g, tc4 * 128:(tc4 + 1) * 128],
                                w16[:, g, :],
                                start=(g == 0), stop=(g == HPC - 1),
                            )
                        ot = osb.tile([128, 512], BF16, tag="ot")
                        nc.vector.tensor_copy(ot[:], acc[:])
                        nc.sync.dma_start(
                            half[tc4 * 128:(tc4 + 1) * 128,
                                 dsl * 512:(dsl + 1) * 512],
                            ot[:],
                        )
                    if ds == 4:
                        nc.gpsimd.collective_compute(
                            "ReduceScatter", mybir.AluOpType.add,
                            replica_groups=RG,
                            ins=[rs_in_a[:, :].opt()],
                            outs=[rs_out_a[:, :].opt()],
                        )
                        nc.gpsimd.dma_start(
                            out_sh[:, 0:HID // 2], rs_out_a[:, :]
                        )

            nc.gpsimd.collective_compute(
                "ReduceScatter", mybir.AluOpType.add,
                replica_groups=RG,
                ins=[rs_in_b[:, :].opt()], outs=[rs_out_b[:, :].opt()],
            )
            nc.gpsimd.dma_start(out_sh[:, HID // 2:], rs_out_b[:, :])

    nc.compile()
    return nc


_CACHE = {}


def _get_program(consts):
    key = (consts["cos2"].tobytes(), consts["sin2"].tobytes())
    if key not in _CACHE:
        _CACHE[key] = _build_program(consts)
    return _CACHE[key]


def _run(inputs, **kwargs):
    in_maps, consts = _host_prepare(inputs)
    nc = _get_program(consts)
    res = run_bass_kernel_spmd(nc, in_maps, core_ids=list(range(NCORES)), **kwargs)
    shards = [res.results[c]["out_shard"] for c in range(NCORES)]
    out = np.concatenate(shards, axis=0)[None].astype(np.float32)
    return out, res


def kernel(**inputs) -> np.ndarray:
    return _run(inputs)[0]
